# revision 1
# baseline (speedup 1.0000x reference)
"""Trainium2 Bass kernel for causal ("FORWARD" direction) multi-head attention.

Reference computation (per batch b, n_heads=8, d=128):
  Q = x @ Wq.T ; K = x @ Wk.T ; V = x @ Wv.T          (nn.Linear, no bias)
  scores[h,i,j] = (Qh[i] . Kh[j]) / sqrt(d)
  scores += -10000 where j <= i   (keeps strict upper triangle j > i)
  attn = softmax(scores, axis=j) ; out = attn @ Vh ; concat heads
  Row i=1023 is fully masked; jax softmax's max-subtraction makes it equal
  softmax of the *raw* scores, so the kernel keeps column i=1023 unmasked.

Sharding: data-parallel over batch B=8 -> 8 cores, no collectives.

Device layout (per core, everything transposed so the softmax reduction is a
matmul-friendly partition-dim reduction):
  xT[k,t]       : x.T                                  [1024,1024]
  qT/kT[o,t]    : per head-group of 4 heads            via Wq.T/Wk.T as lhsT
  v[t,o]        : natural V                            via xT as lhsT
  S_T[j,i]      = kT_tile.T @ qT  (contraction over d=128, single tile)
  expS          = exp(S_T + adder)   (adder patterns precomputed on host)
  U_T[dd,i]     = sum_j V[j,dd] expS[j,i]   (matmul accum over j tiles)
  colsum[*,i]   = ones.T @ expS             (partition-broadcast row of sums)
  out_T         = U_T * reciprocal(colsum) -> DRAM (host transposes back)

Scheduling: projections for the NEXT head-group are emitted interleaved with
attention of the current group, so PE fills the gaps where it would otherwise
wait on Activation (exp) results.
"""

import os
import sys
from collections import deque

import numpy as np

if "/opt/trn_rl_repo" not in sys.path:
    sys.path.insert(0, "/opt/trn_rl_repo")

B, T, D, H, DH = 8, 1024, 1024, 8, 128
P = 128          # partition tile
NI = 512         # i-chunk (moving free size)
NG, GH = 2, 4    # head groups x heads per group
NKT = T // P     # 8 contraction tiles
USE_F32R = os.environ.get("KERNEL_MM_DT", "f32r") == "f32r"

_PROGRAM = None  # cached compiled Bass program


def _adder_patterns() -> np.ndarray:
    """[128, 8*512] f32. Blocks 0..3: adders for tiles (jt, ic=0); blocks
    4..7: adders for tiles (jt, ic=1) with column i=1023 left unmasked."""
    ad = np.zeros((P, 8, NI), np.float32)
    j = np.arange(P)
    i = np.arange(NI)
    for jt in range(4):
        ad[:, jt, :] = np.where((P * jt + j)[:, None] <= i[None, :], -10000.0, 0.0)
    for jt in range(4, 8):
        blk = np.where((P * jt + j)[:, None] <= (NI + i)[None, :], -10000.0, 0.0)
        blk[:, NI - 1] = 0.0  # column i=1023 stays raw
        ad[:, jt, :] = blk
    return np.ascontiguousarray(ad.reshape(P, 8 * NI))


def build_program(use_f32r: bool = USE_F32R, compile: bool = True, reps: int = 1):
    import concourse.bass as bass  # noqa: F401
    import concourse.tile as tile
    from concourse import bacc, mybir

    f32 = mybir.dt.float32
    mdt = mybir.dt.float32r if use_f32r else mybir.dt.float32
    Exp = mybir.ActivationFunctionType.Exp
    Copy = mybir.ActivationFunctionType.Copy
    ADD = mybir.AluOpType.add
    MUL = mybir.AluOpType.mult

    nc = bacc.Bacc(
        "TRN2",
        target_bir_lowering=False,
        debug=False,
        enable_asserts=False,
        num_devices=B,
    )

    xT_d = nc.dram_tensor("xT", [D, T], mdt, kind="ExternalInput")
    wq_d = nc.dram_tensor("wqT", [D, D], mdt, kind="ExternalInput")
    wk_d = nc.dram_tensor("wkT", [D, D], mdt, kind="ExternalInput")
    wv_d = nc.dram_tensor("wvT", [D, D], mdt, kind="ExternalInput")
    ad_d = nc.dram_tensor("adders", [P, 8 * NI], f32, kind="ExternalInput")
    on_d = nc.dram_tensor("ones_t", [P, P], mdt, kind="ExternalInput")
    # out is stored TRANSPOSED ([D, T]); the host wrapper transposes back.
    out_d = nc.dram_tensor("out", [D, T], f32, kind="ExternalOutput")

    with tile.TileContext(nc) as tc:
        with (
            tc.tile_pool(name="sb", bufs=1) as sb,
            tc.tile_pool(name="ps", bufs=1, space="PSUM") as ps,
        ):
            KT_ORDER = (4, 0, 5, 1, 6, 2, 7, 3)

            def emit():
                # ---------------- resident loads ----------------
                # xT split across the SP and ACT DMA queues; weights on
                # Pool/SP; adder blocks trickle in on Pool in first-use
                # order.  kt loops consume in KT_ORDER = arrival order.
                xT = [None] * NKT

                def load_xT(k, eng):
                    t = sb.tile([P, T], mdt, tag=f"xT{k}", name=f"xT{k}")
                    eng.dma_start(t[:], xT_d.ap()[P * k : P * (k + 1), :])
                    xT[k] = t

                for k in (4, 0, 5, 1):
                    load_xT(k, nc.sync)
                for k in (6, 2, 7, 3):
                    load_xT(k, nc.scalar)
                ad = [None] * 8
                for jt in range(4):  # ic=0 crossing blocks, needed first
                    t = sb.tile([P, NI], f32, tag=f"ad{jt}", name=f"ad{jt}")
                    nc.scalar.dma_start(t[:], ad_d.ap()[:, NI * jt : NI * (jt + 1)])
                    ad[jt] = t

                copy_flip = [0]

                def psum_to_sbuf(dst_ap, src_ap, eng=None):
                    if eng is None:
                        eng = nc.scalar if copy_flip[0] % 2 == 0 else nc.vector
                        copy_flip[0] += 1
                    if eng is nc.scalar:
                        eng.activation(dst_ap, src_ap, Copy)
                    else:
                        eng.tensor_copy(dst_ap, src_ap)


                W, QKV = {}, {}

                def load_weights(g):
                    dram = {"wq": wq_d, "wk": wk_d, "wv": wv_d}
                    if g == 0:
                        placement = {
                            "wq": [(kt, nc.gpsimd) for kt in KT_ORDER],
                            "wk": [(4, nc.sync), (0, nc.sync), (5, nc.sync),
                                   (1, nc.sync), (6, nc.gpsimd), (2, nc.gpsimd),
                                   (7, nc.gpsimd), (3, nc.gpsimd)],
                            "wv": [(4, nc.sync), (0, nc.sync), (5, nc.sync),
                                   (1, nc.sync), (6, nc.gpsimd), (2, nc.gpsimd),
                                   (7, nc.gpsimd), (3, nc.gpsimd)],
                        }
                        order = ("wq", "wk", "wv")
                    else:
                        placement = {
                            nm: [(kt, nc.sync) for kt in KT_ORDER]
                            for nm in ("wv", "wq", "wk")
                        }
                        order = ("wv", "wq", "wk")
                    lists = {}
                    for nm in order:
                        lst = [None] * NKT
                        for kt, eng in placement[nm]:
                            w = sb.tile(
                                [P, NI], mdt, tag=f"{nm}{kt}", name=f"{nm}{kt}g{g}"
                            )
                            eng.dma_start(
                                w[:],
                                dram[nm].ap()[P * kt : P * (kt + 1), NI * g : NI * (g + 1)],
                            )
                            lst[kt] = w
                        lists[nm] = lst
                    W[g] = (lists["wq"], lists["wk"], lists["wv"])
                    QKV[g] = (
                        [
                            sb.tile([P, T], mdt, tag=f"qT{ot}", name=f"qT{ot}g{g}")
                            for ot in range(GH)
                        ],
                        [
                            sb.tile([P, T], mdt, tag=f"kT{ot}", name=f"kT{ot}g{g}")
                            for ot in range(GH)
                        ],
                        [
                            sb.tile([P, NI], mdt, tag=f"v{tt}", bufs=2, name=f"v{tt}g{g}")
                            for tt in range(NKT)
                        ],
                    )

                # ---------------- projection generators ----------------
                def proj_qk_gen(g, ot, copy_eng=None):
                    wq_g, wk_g, _ = W[g]
                    qT_g, kT_g, _ = QKV[g]
                    for wlist, dst in ((wq_g, qT_g[ot]), (wk_g, kT_g[ot])):
                        for tci in range(2):
                            pp = ps.tile([P, NI], f32, tag="pp", bufs=2, name="pp")
                            for ki, kt in enumerate(KT_ORDER):
                                nc.tensor.matmul(
                                    pp[:],
                                    wlist[kt][:, P * ot : P * (ot + 1)],
                                    xT[kt][:, NI * tci : NI * (tci + 1)],
                                    start=(ki == 0),
                                    stop=(ki == NKT - 1),
                                )
                            psum_to_sbuf(
                                dst[:, NI * tci : NI * (tci + 1)], pp[:], copy_eng
                            )
                            yield

                def proj_v_gen(g):
                    _, _, wv_g = W[g]
                    _, _, v_g = QKV[g]
                    for tt in range(NKT):
                        pp = ps.tile([P, NI], f32, tag="pp", bufs=2, name="pp")
                        for ki, kt in enumerate(KT_ORDER):
                            nc.tensor.matmul(
                                pp[:],
                                xT[kt][:, P * tt : P * (tt + 1)],
                                wv_g[kt][:],
                                start=(ki == 0),
                                stop=(ki == NKT - 1),
                            )
                        psum_to_sbuf(v_g[tt][:], pp[:])
                        yield

                def wload_gen(g):
                    load_weights(g)
                    return
                    yield  # noqa: unreachable - makes this a generator

                # ---------------- attention generator ----------------
                def attn_gen(g, ot):
                    h = GH * g + ot
                    last_unit = g == NG - 1 and ot == GH - 1
                    qT_g, kT_g, v_g = QKV[g]
                    qh, kh = qT_g[ot], kT_g[ot]

                    # HW rejects fp32r matmuls with tiny output free size
                    # (s3d3_mm_fp32r_restrictions); run those as plain fp32.
                    def smallmm(ap):
                        return ap.bitcast(f32) if use_f32r else ap

                    # last unit runs ic=1 first: ic=0 has no exception path,
                    # so the end-of-program dependency tail is shorter
                    ic_order = (1, 0) if last_unit else (0, 1)
                    for ic in ic_order:
                        jts = list(range(8)) if ic == 0 else [4, 5, 6, 7]
                        nj = len(jts)

                        u_ps = ps.tile([P, NI], f32, tag="u", bufs=2, name="u_ps")
                        c_ps = ps.tile([P, NI], f32, tag="c", bufs=1, name="c_ps")

                        col_ps = colE = None
                        if ic == 1:
                            # raw scores for column i=1023, rows j in [0,512)
                            col_ps = ps.tile([P, 8], f32, tag="col", bufs=1, name="col_ps")
                            for jc in range(4):
                                nc.tensor.matmul(
                                    col_ps[:, jc : jc + 1],
                                    smallmm(kh[:, P * jc : P * (jc + 1)]),
                                    smallmm(qh[:, T - 1 : T]),
                                    start=True,
                                    stop=True,
                                )
                            colE = sb.tile([P, 8], mdt, tag="colE", bufs=2, name="colE")
                            nc.scalar.activation(colE[:, 0:4], col_ps[:, 0:4], Exp)

                        pend = []
                        eacc = [None]

                        def drain_one():
                            idx, jt, e_sb = pend.pop(0)
                            first, last = idx == 0, idx == nj - 1
                            nc.tensor.matmul(
                                u_ps[:],
                                v_g[jt][:, P * ot : P * (ot + 1)],
                                e_sb[:],
                                start=first,
                                stop=last,
                            )
                            # colsum via elementwise tile accumulation (DVE);
                            # one ones-matmul at the end reduces partitions.
                            # SBUF-only chain -> Pool (gpsimd can't touch PSUM)
                            if idx == 0:
                                eacc[0] = e_sb
                            elif idx == 1:
                                acc = sb.tile(
                                    [P, NI], mdt, tag="eacc", bufs=2, name="eacc"
                                )
                                nc.gpsimd.tensor_tensor(
                                    acc[:], eacc[0][:], e_sb[:], ADD
                                )
                                eacc[0] = acc
                            else:
                                nc.gpsimd.tensor_tensor(
                                    eacc[0][:], eacc[0][:], e_sb[:], ADD
                                )

                        for idx, jt in enumerate(jts):
                            # last unit has no proj filler: borrow the idle pp
                            # psum banks to deepen the S pipeline
                            stag = "pp" if (last_unit and idx % 2 == 1) else "s"
                            s_ps = ps.tile([P, NI], f32, tag=stag, bufs=2, name="s_ps")
                            nc.tensor.matmul(
                                s_ps[:],
                                kh[:, P * jt : P * (jt + 1)],
                                qh[:, NI * ic : NI * (ic + 1)],
                                start=True,
                                stop=True,
                            )
                            crossing = (ic == 0 and jt < 4) or (ic == 1 and jt >= 4)
                            if crossing:
                                nc.vector.tensor_tensor(
                                    s_ps[:], s_ps[:], ad[jt][:], ADD
                                )
                            e_sb = sb.tile([P, NI], mdt, tag="e", bufs=6, name="e_sb")
                            nc.scalar.activation(e_sb[:], s_ps[:], Exp)
                            pend.append((idx, jt, e_sb))
                            while len(pend) > 3:
                                drain_one()
                            yield
                        while pend:
                            drain_one()
                        nc.tensor.matmul(
                            c_ps[:], ones[:], eacc[0][:], start=True, stop=True
                        )

                        if ic == 1:
                            # fold the j<512 contributions of column i=1023 in
                            for jc in range(4):
                                nc.tensor.matmul(
                                    col_ps[:, 4:5],
                                    smallmm(v_g[jc][:, P * ot : P * (ot + 1)]),
                                    smallmm(colE[:, jc : jc + 1]),
                                    start=(jc == 0),
                                    stop=(jc == 3),
                                )
                            for jc in range(4):
                                nc.tensor.matmul(
                                    col_ps[:, 5:6],
                                    smallmm(ones[:]),
                                    smallmm(colE[:, jc : jc + 1]),
                                    start=(jc == 0),
                                    stop=(jc == 3),
                                )
                            colsb = sb.tile([P, 2], f32, tag="colsb", bufs=2, name="colsb")
                            nc.scalar.activation(colsb[:], col_ps[:, 4:6], Copy)
                            nc.vector.tensor_tensor(
                                u_ps[:, NI - 1 : NI], u_ps[:, NI - 1 : NI], colsb[:, 0:1], ADD
                            )
                            nc.vector.tensor_tensor(
                                c_ps[:, NI - 1 : NI], c_ps[:, NI - 1 : NI], colsb[:, 1:2], ADD
                            )

                        recip = sb.tile([P, NI], f32, tag="recip", bufs=2, name="recip")
                        o_sb = sb.tile([P, NI], f32, tag="o", bufs=3, name="o_sb")
                        if last_unit and ic == 0:
                            # final epilogue is fully exposed: halve the DVE
                            # chain so the first out-DMA overlaps the second
                            hn = NI // 2
                            for hf in range(2):
                                sl = slice(hn * hf, hn * (hf + 1))
                                nc.vector.reciprocal(recip[:, sl], c_ps[:, sl])
                                nc.vector.tensor_tensor(
                                    o_sb[:, sl], u_ps[:, sl], recip[:, sl], MUL
                                )
                                nc.sync.dma_start(
                                    out_d.ap()[
                                        P * h : P * (h + 1),
                                        NI * ic + hn * hf : NI * ic + hn * (hf + 1),
                                    ],
                                    o_sb[:, sl],
                                )
                        else:
                            nc.vector.reciprocal(recip[:], c_ps[:])
                            nc.vector.tensor_tensor(o_sb[:], u_ps[:], recip[:], MUL)
                            nc.sync.dma_start(
                                out_d.ap()[P * h : P * (h + 1), NI * ic : NI * (ic + 1)],
                                o_sb[:],
                            )
                        yield

                # ---------------- schedule ----------------
                load_weights(0)
                ones = sb.tile([P, P], mdt, tag="ones", name="ones")
                nc.gpsimd.dma_start(ones[:], on_d.ap()[:])
                for jt in range(4, 8):  # ic=1 crossing blocks, needed later
                    t = sb.tile([P, NI], f32, tag=f"ad{jt}", name=f"ad{jt}")
                    nc.gpsimd.dma_start(t[:], ad_d.ap()[:, NI * jt : NI * (jt + 1)])
                    ad[jt] = t
                for _ in proj_qk_gen(0, 0, copy_eng=nc.vector):
                    pass

                # Filler generators are window-scoped: proj work for (g1, ot)
                # may only be emitted strictly after attn(g0, ot) has finished
                # emitting (WAR hazards on the single-buffered qT/kT/w tiles
                # would otherwise deadlock the in-order engine queues).
                windows = {
                    (0, 0): [proj_v_gen(0), proj_qk_gen(0, 1)],
                    (0, 1): [proj_qk_gen(0, 2)],
                    (0, 2): [proj_qk_gen(0, 3), wload_gen(1), proj_qk_gen(1, 0)],
                    (0, 3): [proj_v_gen(1)],
                    (1, 0): [proj_qk_gen(1, 1)],
                    (1, 1): [proj_qk_gen(1, 2)],
                    (1, 2): [proj_qk_gen(1, 3)],
                }

                for g in range(NG):
                    for ot in range(GH):
                        filler = deque(windows.get((g, ot), []))

                        def pump(n):
                            while n > 0 and filler:
                                try:
                                    next(filler[0])
                                    n -= 1
                                except StopIteration:
                                    filler.popleft()

                        for _ in attn_gen(g, ot):
                            pump(1)
                        pump(10**9)  # drain before the next unit starts

            for _rep in range(reps):
                emit()

    if compile:
        nc.compile()
    return nc


def _get_program():
    global _PROGRAM
    if _PROGRAM is None:
        _PROGRAM = build_program()
    return _PROGRAM


def make_in_maps(x, Wq, Wk, Wv):
    scale = 1.0 / np.sqrt(np.float32(DH))
    wqT = np.ascontiguousarray(np.asarray(Wq, np.float32).T * scale)
    wkT = np.ascontiguousarray(np.asarray(Wk, np.float32).T)
    wvT = np.ascontiguousarray(np.asarray(Wv, np.float32).T)
    adders = _adder_patterns()
    ones = np.ones((P, P), np.float32)
    x = np.asarray(x, np.float32)
    in_maps = []
    for b in range(B):
        in_maps.append(
            {
                "xT": np.ascontiguousarray(x[b].T),
                "wqT": wqT,
                "wkT": wkT,
                "wvT": wvT,
                "adders": adders,
                "ones_t": ones,
            }
        )
    return in_maps


def kernel(x, mask, Wq, Wk, Wv, _trace=False):
    from concourse.bass_utils import run_bass_kernel_spmd

    nc = _get_program()
    in_maps = make_in_maps(x, Wq, Wk, Wv)
    res = run_bass_kernel_spmd(nc, in_maps, core_ids=list(range(B)), trace=_trace)
    out = np.stack([res.results[b]["out"] for b in range(B)], axis=0)
    out = np.swapaxes(out, 1, 2)  # device stores out.T
    out = out * np.asarray(mask, np.float32)[:, :, None]
    out = np.ascontiguousarray(out, np.float32)
    if _trace:
        kernel.last_results = res
    return out



# revision 5
# speedup vs baseline: 258.6510x; 258.6510x over previous
"""Trainium2 Bass kernel for causal ("FORWARD" direction) multi-head attention.

Reference computation (per batch b, n_heads=8, d=128):
  Q = x @ Wq.T ; K = x @ Wk.T ; V = x @ Wv.T          (nn.Linear, no bias)
  scores[h,i,j] = (Qh[i] . Kh[j]) / sqrt(d)
  scores += -10000 where j <= i   (keeps strict upper triangle j > i)
  attn = softmax(scores, axis=j) ; out = attn @ Vh ; concat heads
  Row i=1023 is fully masked; jax softmax's max-subtraction makes it equal
  softmax of the *raw* scores, so the kernel keeps column i=1023 unmasked.

Sharding: data-parallel over batch B=8 -> 8 cores, no collectives.

Device layout (per core, everything transposed so the softmax reduction is a
matmul-friendly partition-dim reduction):
  xT[k,t]       : x.T                                  [1024,1024]
  qT/kT[o,t]    : per head-group of 4 heads            via Wq.T/Wk.T as lhsT
  v[t,o]        : natural V                            via xT as lhsT
  S_T[j,i]      = kT_tile.T @ qT  (contraction over d=128, single tile)
  expS          = exp(S_T + adder)   (adder patterns precomputed on host)
  U_T[dd,i]     = sum_j V[j,dd] expS[j,i]   (matmul accum over j tiles)
  colsum[*,i]   = ones.T @ expS             (partition-broadcast row of sums)
  out_T         = U_T * reciprocal(colsum) -> DRAM (host transposes back)

Scheduling: projections for the NEXT head-group are emitted interleaved with
attention of the current group, so PE fills the gaps where it would otherwise
wait on Activation (exp) results.
"""

import os
import sys
from collections import deque

import numpy as np

if "/opt/trn_rl_repo" not in sys.path:
    sys.path.insert(0, "/opt/trn_rl_repo")

B, T, D, H, DH = 8, 1024, 1024, 8, 128
P = 128          # partition tile
NI = 512         # i-chunk (moving free size)
NG, GH = 2, 4    # head groups x heads per group
NKT = T // P     # 8 contraction tiles
# f32r (fp32 bits, full-rate matmul streaming) is the default: measured on
# this hardware, bf16 matmuls stream at the same 1 column/cycle rate as
# f32r (no 2x), so bf16 would only add quantization error.
MM_DT = os.environ.get("KERNEL_MM_DT", "f32r")  # f32r | f32 | bf16

_PROGRAM = None  # cached compiled Bass program


def _adder_patterns() -> np.ndarray:
    """[128, 8*512] f32. Blocks 0..3: adders for tiles (jt, ic=0); blocks
    4..7: adders for tiles (jt, ic=1) with column i=1023 left unmasked."""
    ad = np.zeros((P, 8, NI), np.float32)
    j = np.arange(P)
    i = np.arange(NI)
    for jt in range(4):
        ad[:, jt, :] = np.where((P * jt + j)[:, None] <= i[None, :], -10000.0, 0.0)
    for jt in range(4, 8):
        blk = np.where((P * jt + j)[:, None] <= (NI + i)[None, :], -10000.0, 0.0)
        blk[:, NI - 1] = 0.0  # column i=1023 stays raw
        ad[:, jt, :] = blk
    return np.ascontiguousarray(ad.reshape(P, 8 * NI))


def build_program(mm_dt: str = MM_DT, compile: bool = True, reps: int = 1):
    import concourse.bass as bass  # noqa: F401
    import concourse.tile as tile
    from concourse import bacc, mybir

    f32 = mybir.dt.float32
    use_f32r = mm_dt == "f32r"
    mdt = {
        "bf16": mybir.dt.bfloat16,
        "f32r": mybir.dt.float32r,
        "f32": mybir.dt.float32,
    }[mm_dt]
    Exp = mybir.ActivationFunctionType.Exp
    Copy = mybir.ActivationFunctionType.Copy
    ADD = mybir.AluOpType.add
    MUL = mybir.AluOpType.mult

    nc = bacc.Bacc(
        "TRN2",
        target_bir_lowering=False,
        debug=False,
        enable_asserts=False,
        num_devices=B,
    )

    xT_d = nc.dram_tensor("xT", [D, T], mdt, kind="ExternalInput")
    wq_d = nc.dram_tensor("wqT", [D, D], mdt, kind="ExternalInput")
    wk_d = nc.dram_tensor("wkT", [D, D], mdt, kind="ExternalInput")
    wv_d = nc.dram_tensor("wvT", [D, D], mdt, kind="ExternalInput")
    ad_d = nc.dram_tensor("adders", [P, 8 * NI], f32, kind="ExternalInput")
    on_d = nc.dram_tensor("ones_t", [P, P], mdt, kind="ExternalInput")
    # out is stored TRANSPOSED ([D, T]); the host wrapper transposes back.
    out_d = nc.dram_tensor("out", [D, T], f32, kind="ExternalOutput")

    with tile.TileContext(nc) as tc:
        with (
            tc.tile_pool(name="sb", bufs=1) as sb,
            tc.tile_pool(name="ps", bufs=1, space="PSUM") as ps,
        ):
            KT_ORDER = (4, 0, 5, 1, 6, 2, 7, 3)

            def emit():
                # ---------------- resident loads ----------------
                # xT split across the SP and ACT DMA queues; weights on
                # Pool/SP; adder blocks trickle in on Pool in first-use
                # order.  kt loops consume in KT_ORDER = arrival order.
                xT = [None] * NKT

                def load_xT(k, eng):
                    t = sb.tile([P, T], mdt, tag=f"xT{k}", name=f"xT{k}")
                    eng.dma_start(t[:], xT_d.ap()[P * k : P * (k + 1), :])
                    xT[k] = t

                for k in (4, 0, 5, 1):
                    load_xT(k, nc.sync)
                for k in (6, 2, 7, 3):
                    load_xT(k, nc.scalar)
                ad = [None] * 8
                for jt in range(4):  # ic=0 crossing blocks, needed first
                    t = sb.tile([P, NI], f32, tag=f"ad{jt}", name=f"ad{jt}")
                    nc.scalar.dma_start(t[:], ad_d.ap()[:, NI * jt : NI * (jt + 1)])
                    ad[jt] = t

                copy_flip = [0]

                def psum_to_sbuf(dst_ap, src_ap, eng=None):
                    if eng is None:
                        eng = nc.scalar if copy_flip[0] % 2 == 0 else nc.vector
                        copy_flip[0] += 1
                    if eng is nc.scalar:
                        eng.activation(dst_ap, src_ap, Copy)
                    else:
                        eng.tensor_copy(dst_ap, src_ap)


                W, QKV = {}, {}

                def load_weights(g):
                    dram = {"wq": wq_d, "wk": wk_d, "wv": wv_d}
                    if g == 0:
                        placement = {
                            "wq": [(kt, nc.gpsimd) for kt in KT_ORDER],
                            "wk": [(4, nc.sync), (0, nc.sync), (5, nc.sync),
                                   (1, nc.sync), (6, nc.gpsimd), (2, nc.gpsimd),
                                   (7, nc.gpsimd), (3, nc.gpsimd)],
                            "wv": [(4, nc.sync), (0, nc.sync), (5, nc.sync),
                                   (1, nc.sync), (6, nc.gpsimd), (2, nc.gpsimd),
                                   (7, nc.gpsimd), (3, nc.gpsimd)],
                        }
                        order = ("wq", "wk", "wv")
                    else:
                        placement = {
                            nm: [(kt, nc.sync) for kt in KT_ORDER]
                            for nm in ("wv", "wq", "wk")
                        }
                        order = ("wv", "wq", "wk")
                    lists = {}
                    for nm in order:
                        lst = [None] * NKT
                        for kt, eng in placement[nm]:
                            w = sb.tile(
                                [P, NI], mdt, tag=f"{nm}{kt}", name=f"{nm}{kt}g{g}"
                            )
                            eng.dma_start(
                                w[:],
                                dram[nm].ap()[P * kt : P * (kt + 1), NI * g : NI * (g + 1)],
                            )
                            lst[kt] = w
                        lists[nm] = lst
                    W[g] = (lists["wq"], lists["wk"], lists["wv"])
                    QKV[g] = (
                        [
                            sb.tile([P, T], mdt, tag=f"qT{ot}", name=f"qT{ot}g{g}")
                            for ot in range(GH)
                        ],
                        [
                            sb.tile([P, T], mdt, tag=f"kT{ot}", name=f"kT{ot}g{g}")
                            for ot in range(GH)
                        ],
                        [
                            sb.tile([P, NI], mdt, tag=f"v{tt}", bufs=2, name=f"v{tt}g{g}")
                            for tt in range(NKT)
                        ],
                    )

                # ---------------- projection generators ----------------
                def proj_qk_gen(g, ot, copy_eng=None):
                    wq_g, wk_g, _ = W[g]
                    qT_g, kT_g, _ = QKV[g]
                    for wlist, dst in ((wq_g, qT_g[ot]), (wk_g, kT_g[ot])):
                        for tci in range(2):
                            pp = ps.tile([P, NI], f32, tag="pp", bufs=2, name="pp")
                            for ki, kt in enumerate(KT_ORDER):
                                nc.tensor.matmul(
                                    pp[:],
                                    wlist[kt][:, P * ot : P * (ot + 1)],
                                    xT[kt][:, NI * tci : NI * (tci + 1)],
                                    start=(ki == 0),
                                    stop=(ki == NKT - 1),
                                )
                            psum_to_sbuf(
                                dst[:, NI * tci : NI * (tci + 1)], pp[:], copy_eng
                            )
                            yield

                def proj_v_gen(g):
                    _, _, wv_g = W[g]
                    _, _, v_g = QKV[g]
                    for tt in range(NKT):
                        pp = ps.tile([P, NI], f32, tag="pp", bufs=2, name="pp")
                        for ki, kt in enumerate(KT_ORDER):
                            nc.tensor.matmul(
                                pp[:],
                                xT[kt][:, P * tt : P * (tt + 1)],
                                wv_g[kt][:],
                                start=(ki == 0),
                                stop=(ki == NKT - 1),
                            )
                        psum_to_sbuf(v_g[tt][:], pp[:])
                        yield

                def wload_gen(g):
                    load_weights(g)
                    return
                    yield  # noqa: unreachable - makes this a generator

                # ---------------- attention generator ----------------
                def attn_gen(g, ot):
                    h = GH * g + ot
                    last_unit = g == NG - 1 and ot == GH - 1
                    qT_g, kT_g, v_g = QKV[g]
                    qh, kh = qT_g[ot], kT_g[ot]

                    # HW rejects fp32r matmuls with tiny output free size
                    # (s3d3_mm_fp32r_restrictions); run those as plain fp32.
                    def smallmm(ap):
                        return ap.bitcast(f32) if use_f32r else ap

                    # last unit runs ic=1 first: ic=0 has no exception path,
                    # so the end-of-program dependency tail is shorter
                    ic_order = (1, 0) if last_unit else (0, 1)
                    for ic in ic_order:
                        jts = list(range(8)) if ic == 0 else [4, 5, 6, 7]
                        nj = len(jts)

                        u_ps = ps.tile([P, NI], f32, tag="u", bufs=2, name="u_ps")
                        c_ps = ps.tile([P, NI], f32, tag="c", bufs=1, name="c_ps")

                        col_ps = colE = None
                        if ic == 1:
                            # raw scores for column i=1023, rows j in [0,512)
                            col_ps = ps.tile([P, 8], f32, tag="col", bufs=1, name="col_ps")
                            for jc in range(4):
                                nc.tensor.matmul(
                                    col_ps[:, jc : jc + 1],
                                    smallmm(kh[:, P * jc : P * (jc + 1)]),
                                    smallmm(qh[:, T - 1 : T]),
                                    start=True,
                                    stop=True,
                                )
                            colE = sb.tile([P, 8], mdt, tag="colE", bufs=2, name="colE")
                            nc.scalar.activation(colE[:, 0:4], col_ps[:, 0:4], Exp)

                        pend = []
                        eacc = [None]

                        def drain_one():
                            idx, jt, e_sb = pend.pop(0)
                            first, last = idx == 0, idx == nj - 1
                            nc.tensor.matmul(
                                u_ps[:],
                                v_g[jt][:, P * ot : P * (ot + 1)],
                                e_sb[:],
                                start=first,
                                stop=last,
                            )
                            # colsum via elementwise tile accumulation (DVE);
                            # one ones-matmul at the end reduces partitions.
                            # SBUF-only chain -> Pool (gpsimd can't touch PSUM)
                            if idx == 0:
                                eacc[0] = e_sb
                            elif idx == 1:
                                acc = sb.tile(
                                    [P, NI], mdt, tag="eacc", bufs=2, name="eacc"
                                )
                                nc.gpsimd.tensor_tensor(
                                    acc[:], eacc[0][:], e_sb[:], ADD
                                )
                                eacc[0] = acc
                            else:
                                nc.gpsimd.tensor_tensor(
                                    eacc[0][:], eacc[0][:], e_sb[:], ADD
                                )

                        for idx, jt in enumerate(jts):
                            # last unit has no proj filler: borrow the idle pp
                            # psum banks to deepen the S pipeline
                            stag = "pp" if (last_unit and idx % 2 == 1) else "s"
                            s_ps = ps.tile([P, NI], f32, tag=stag, bufs=2, name="s_ps")
                            nc.tensor.matmul(
                                s_ps[:],
                                kh[:, P * jt : P * (jt + 1)],
                                qh[:, NI * ic : NI * (ic + 1)],
                                start=True,
                                stop=True,
                            )
                            crossing = (ic == 0 and jt < 4) or (ic == 1 and jt >= 4)
                            if crossing:
                                nc.vector.tensor_tensor(
                                    s_ps[:], s_ps[:], ad[jt][:], ADD
                                )
                            e_sb = sb.tile([P, NI], mdt, tag="e", bufs=6, name="e_sb")
                            nc.scalar.activation(e_sb[:], s_ps[:], Exp)
                            pend.append((idx, jt, e_sb))
                            while len(pend) > 3:
                                drain_one()
                            yield
                        while pend:
                            drain_one()
                        nc.tensor.matmul(
                            c_ps[:], ones[:], eacc[0][:], start=True, stop=True
                        )

                        if ic == 1:
                            # fold the j<512 contributions of column i=1023 in
                            for jc in range(4):
                                nc.tensor.matmul(
                                    col_ps[:, 4:5],
                                    smallmm(v_g[jc][:, P * ot : P * (ot + 1)]),
                                    smallmm(colE[:, jc : jc + 1]),
                                    start=(jc == 0),
                                    stop=(jc == 3),
                                )
                            for jc in range(4):
                                nc.tensor.matmul(
                                    col_ps[:, 5:6],
                                    smallmm(ones[:]),
                                    smallmm(colE[:, jc : jc + 1]),
                                    start=(jc == 0),
                                    stop=(jc == 3),
                                )
                            colsb = sb.tile([P, 2], f32, tag="colsb", bufs=2, name="colsb")
                            nc.scalar.activation(colsb[:], col_ps[:, 4:6], Copy)
                            nc.vector.tensor_tensor(
                                u_ps[:, NI - 1 : NI], u_ps[:, NI - 1 : NI], colsb[:, 0:1], ADD
                            )
                            nc.vector.tensor_tensor(
                                c_ps[:, NI - 1 : NI], c_ps[:, NI - 1 : NI], colsb[:, 1:2], ADD
                            )

                        recip = sb.tile([P, NI], f32, tag="recip", bufs=2, name="recip")
                        o_sb = sb.tile([P, NI], f32, tag="o", bufs=3, name="o_sb")
                        if last_unit and ic == 0:
                            # final epilogue is fully exposed: halve the DVE
                            # chain so the first out-DMA overlaps the second
                            hn = NI // 2
                            for hf in range(2):
                                sl = slice(hn * hf, hn * (hf + 1))
                                nc.vector.reciprocal(recip[:, sl], c_ps[:, sl])
                                nc.vector.tensor_tensor(
                                    o_sb[:, sl], u_ps[:, sl], recip[:, sl], MUL
                                )
                                nc.sync.dma_start(
                                    out_d.ap()[
                                        P * h : P * (h + 1),
                                        NI * ic + hn * hf : NI * ic + hn * (hf + 1),
                                    ],
                                    o_sb[:, sl],
                                )
                        else:
                            nc.vector.reciprocal(recip[:], c_ps[:])
                            nc.vector.tensor_tensor(o_sb[:], u_ps[:], recip[:], MUL)
                            nc.sync.dma_start(
                                out_d.ap()[P * h : P * (h + 1), NI * ic : NI * (ic + 1)],
                                o_sb[:],
                            )
                        yield

                # ---------------- schedule ----------------
                load_weights(0)
                ones = sb.tile([P, P], mdt, tag="ones", name="ones")
                nc.gpsimd.dma_start(ones[:], on_d.ap()[:])
                for jt in range(4, 8):  # ic=1 crossing blocks, needed later
                    t = sb.tile([P, NI], f32, tag=f"ad{jt}", name=f"ad{jt}")
                    nc.gpsimd.dma_start(t[:], ad_d.ap()[:, NI * jt : NI * (jt + 1)])
                    ad[jt] = t
                for _ in proj_qk_gen(0, 0, copy_eng=nc.vector):
                    pass

                # Filler generators are window-scoped: proj work for (g1, ot)
                # may only be emitted strictly after attn(g0, ot) has finished
                # emitting (WAR hazards on the single-buffered qT/kT/w tiles
                # would otherwise deadlock the in-order engine queues).
                windows = {
                    (0, 0): [proj_v_gen(0), proj_qk_gen(0, 1)],
                    (0, 1): [proj_qk_gen(0, 2)],
                    (0, 2): [proj_qk_gen(0, 3), wload_gen(1), proj_qk_gen(1, 0)],
                    (0, 3): [proj_v_gen(1)],
                    (1, 0): [proj_qk_gen(1, 1)],
                    (1, 1): [proj_qk_gen(1, 2)],
                    (1, 2): [proj_qk_gen(1, 3)],
                }

                for g in range(NG):
                    for ot in range(GH):
                        filler = deque(windows.get((g, ot), []))

                        def pump(n):
                            while n > 0 and filler:
                                try:
                                    next(filler[0])
                                    n -= 1
                                except StopIteration:
                                    filler.popleft()

                        for _ in attn_gen(g, ot):
                            pump(1)
                        pump(10**9)  # drain before the next unit starts

            for _rep in range(reps):
                emit()

    if compile:
        nc.compile()
    return nc


def _get_program():
    global _PROGRAM
    if _PROGRAM is None:
        _PROGRAM = build_program()
    return _PROGRAM


def make_in_maps(x, Wq, Wk, Wv):
    scale = 1.0 / np.sqrt(np.float32(DH))
    wqT = np.ascontiguousarray(np.asarray(Wq, np.float32).T * scale)
    wkT = np.ascontiguousarray(np.asarray(Wk, np.float32).T)
    wvT = np.ascontiguousarray(np.asarray(Wv, np.float32).T)
    adders = _adder_patterns()
    ones = np.ones((P, P), np.float32)
    x = np.asarray(x, np.float32)
    if MM_DT == "bf16":
        import ml_dtypes

        bf16 = ml_dtypes.bfloat16
        wqT, wkT, wvT = wqT.astype(bf16), wkT.astype(bf16), wvT.astype(bf16)
        ones = ones.astype(bf16)
        x = x.astype(bf16)
    in_maps = []
    for b in range(B):
        in_maps.append(
            {
                "xT": np.ascontiguousarray(x[b].T),
                "wqT": wqT,
                "wkT": wkT,
                "wvT": wvT,
                "adders": adders,
                "ones_t": ones,
            }
        )
    return in_maps


def kernel(x, mask, Wq, Wk, Wv, _trace=False):
    from concourse.bass_utils import run_bass_kernel_spmd

    nc = _get_program()
    in_maps = make_in_maps(x, Wq, Wk, Wv)
    res = run_bass_kernel_spmd(nc, in_maps, core_ids=list(range(B)), trace=_trace)
    out = np.stack([res.results[b]["out"] for b in range(B)], axis=0)
    out = np.swapaxes(out, 1, 2)  # device stores out.T
    out = out * np.asarray(mask, np.float32)[:, :, None]
    out = np.ascontiguousarray(out, np.float32)
    if _trace:
        kernel.last_results = res
    return out



# revision 6
# speedup vs baseline: 282.4030x; 1.0918x over previous
"""Trainium2 Bass kernel for causal ("FORWARD" direction) multi-head attention.

Reference computation (per batch b, n_heads=8, d=128):
  Q = x @ Wq.T ; K = x @ Wk.T ; V = x @ Wv.T          (nn.Linear, no bias)
  scores[h,i,j] = (Qh[i] . Kh[j]) / sqrt(d)
  scores += -10000 where j <= i   (keeps strict upper triangle j > i)
  attn = softmax(scores, axis=j) ; out = attn @ Vh ; concat heads
  Row i=1023 is fully masked; jax softmax's max-subtraction makes it equal
  softmax of the *raw* scores, so the kernel keeps column i=1023 unmasked.

Sharding: data-parallel over batch B=8 -> 8 cores, no collectives.

Device layout (per core, everything transposed so the softmax reduction is a
matmul-friendly partition-dim reduction):
  xT[k,t]       : x.T                                  [1024,1024]
  qT/kT[o,t]    : per head-group of 4 heads            via Wq.T/Wk.T as lhsT
  v[t,o]        : natural V                            via xT as lhsT
  S_T[j,i]      = kT_tile.T @ qT  (contraction over d=128, single tile)
  expS          = exp(S_T + adder)   (adder patterns precomputed on host)
  U_T[dd,i]     = sum_j V[j,dd] expS[j,i]   (matmul accum over j tiles)
  colsum[*,i]   = ones.T @ expS             (partition-broadcast row of sums)
  out_T         = U_T * reciprocal(colsum) -> DRAM (host transposes back)

Scheduling: projections for the NEXT head-group are emitted interleaved with
attention of the current group, so PE fills the gaps where it would otherwise
wait on Activation (exp) results.

Performance notes (HW-measured via amortized reps-in-NEFF slope):
  - This hardware streams matmul moving operands at 1 column/cycle @
    ~1.28 GHz for f32r AND bf16 alike (no warm 2.4 GHz state, no 2x bf16
    column rate).  Measured: [128,128]@[128,512] MM = 417.6 ns = pure
    512-column streaming; N=256 -> 259 ns, N=128 -> 287 ns (large fixed
    per-MM overhead below N=512).
  - The kernel issues 592 N=512 matmuls = 303K moving columns ~= 237.4 us
    floor; measured steady-state per-invocation 237.2 us -> zero PE
    bubbles.  Finer-grained causal tiling (N=256/128) LOSES to the per-MM
    overhead; fp8 (DoubleRow) loses to numerics (3.6% per-element noise
    does not average out in random-sign sums -> ~5% output error vs the
    2e-2 gate).  bf16 gains nothing (same column rate) and adds error,
    hence f32r.
"""

import os
import sys
from collections import deque

import numpy as np

if "/opt/trn_rl_repo" not in sys.path:
    sys.path.insert(0, "/opt/trn_rl_repo")

B, T, D, H, DH = 8, 1024, 1024, 8, 128
P = 128          # partition tile
NI = 512         # i-chunk (moving free size)
NG, GH = 2, 4    # head groups x heads per group
NKT = T // P     # 8 contraction tiles
# f32r (fp32 bits, full-rate matmul streaming) is the default: measured on
# this hardware, bf16 matmuls stream at the same 1 column/cycle rate as
# f32r (no 2x), so bf16 would only add quantization error.
MM_DT = os.environ.get("KERNEL_MM_DT", "f32r")  # f32r | f32 | bf16

_PROGRAM = None  # cached compiled Bass program


def _adder_patterns() -> np.ndarray:
    """[128, 8*512] f32. Blocks 0..3: adders for tiles (jt, ic=0); blocks
    4..7: adders for tiles (jt, ic=1) with column i=1023 left unmasked."""
    ad = np.zeros((P, 8, NI), np.float32)
    j = np.arange(P)
    i = np.arange(NI)
    for jt in range(4):
        ad[:, jt, :] = np.where((P * jt + j)[:, None] <= i[None, :], -10000.0, 0.0)
    for jt in range(4, 8):
        blk = np.where((P * jt + j)[:, None] <= (NI + i)[None, :], -10000.0, 0.0)
        blk[:, NI - 1] = 0.0  # column i=1023 stays raw
        ad[:, jt, :] = blk
    return np.ascontiguousarray(ad.reshape(P, 8 * NI))


def build_program(mm_dt: str = MM_DT, compile: bool = True, reps: int = 1):
    import concourse.bass as bass  # noqa: F401
    import concourse.tile as tile
    from concourse import bacc, mybir

    f32 = mybir.dt.float32
    use_f32r = mm_dt == "f32r"
    mdt = {
        "bf16": mybir.dt.bfloat16,
        "f32r": mybir.dt.float32r,
        "f32": mybir.dt.float32,
    }[mm_dt]
    Exp = mybir.ActivationFunctionType.Exp
    Copy = mybir.ActivationFunctionType.Copy
    ADD = mybir.AluOpType.add
    MUL = mybir.AluOpType.mult

    nc = bacc.Bacc(
        "TRN2",
        target_bir_lowering=False,
        debug=False,
        enable_asserts=False,
        num_devices=B,
    )

    xT_d = nc.dram_tensor("xT", [D, T], mdt, kind="ExternalInput")
    wq_d = nc.dram_tensor("wqT", [D, D], mdt, kind="ExternalInput")
    wk_d = nc.dram_tensor("wkT", [D, D], mdt, kind="ExternalInput")
    wv_d = nc.dram_tensor("wvT", [D, D], mdt, kind="ExternalInput")
    ad_d = nc.dram_tensor("adders", [P, 8 * NI], f32, kind="ExternalInput")
    on_d = nc.dram_tensor("ones_t", [P, P], mdt, kind="ExternalInput")
    # out is stored TRANSPOSED ([D, T]); the host wrapper transposes back.
    out_d = nc.dram_tensor("out", [D, T], f32, kind="ExternalOutput")

    with tile.TileContext(nc) as tc:
        with (
            tc.tile_pool(name="sb", bufs=1) as sb,
            tc.tile_pool(name="ps", bufs=1, space="PSUM") as ps,
        ):
            KT_ORDER = (4, 0, 5, 1, 6, 2, 7, 3)

            def emit():
                # ---------------- resident loads ----------------
                # xT split across the SP and ACT DMA queues; weights on
                # Pool/SP; adder blocks trickle in on Pool in first-use
                # order.  kt loops consume in KT_ORDER = arrival order.
                xT = [None] * NKT

                def load_xT(k, eng):
                    t = sb.tile([P, T], mdt, tag=f"xT{k}", name=f"xT{k}")
                    eng.dma_start(t[:], xT_d.ap()[P * k : P * (k + 1), :])
                    xT[k] = t

                for k in (4, 0, 5, 1):
                    load_xT(k, nc.sync)
                for k in (6, 2, 7, 3):
                    load_xT(k, nc.scalar)
                ad = [None] * 8
                for jt in range(4):  # ic=0 crossing blocks, needed first
                    t = sb.tile([P, NI], f32, tag=f"ad{jt}", name=f"ad{jt}")
                    nc.scalar.dma_start(t[:], ad_d.ap()[:, NI * jt : NI * (jt + 1)])
                    ad[jt] = t

                copy_flip = [0]

                def psum_to_sbuf(dst_ap, src_ap, eng=None):
                    if eng is None:
                        eng = nc.scalar if copy_flip[0] % 2 == 0 else nc.vector
                        copy_flip[0] += 1
                    if eng is nc.scalar:
                        eng.activation(dst_ap, src_ap, Copy)
                    else:
                        eng.tensor_copy(dst_ap, src_ap)


                W, QKV = {}, {}

                def load_weights(g):
                    dram = {"wq": wq_d, "wk": wk_d, "wv": wv_d}
                    if g == 0:
                        placement = {
                            "wq": [(kt, nc.gpsimd) for kt in KT_ORDER],
                            "wk": [(4, nc.sync), (0, nc.sync), (5, nc.sync),
                                   (1, nc.sync), (6, nc.gpsimd), (2, nc.gpsimd),
                                   (7, nc.gpsimd), (3, nc.gpsimd)],
                            "wv": [(4, nc.sync), (0, nc.sync), (5, nc.sync),
                                   (1, nc.sync), (6, nc.gpsimd), (2, nc.gpsimd),
                                   (7, nc.gpsimd), (3, nc.gpsimd)],
                        }
                        order = ("wq", "wk", "wv")
                    else:
                        placement = {
                            nm: [(kt, nc.sync) for kt in KT_ORDER]
                            for nm in ("wv", "wq", "wk")
                        }
                        order = ("wv", "wq", "wk")
                    lists = {}
                    for nm in order:
                        lst = [None] * NKT
                        for kt, eng in placement[nm]:
                            w = sb.tile(
                                [P, NI], mdt, tag=f"{nm}{kt}", name=f"{nm}{kt}g{g}"
                            )
                            eng.dma_start(
                                w[:],
                                dram[nm].ap()[P * kt : P * (kt + 1), NI * g : NI * (g + 1)],
                            )
                            lst[kt] = w
                        lists[nm] = lst
                    W[g] = (lists["wq"], lists["wk"], lists["wv"])
                    QKV[g] = (
                        [
                            sb.tile([P, T], mdt, tag=f"qT{ot}", name=f"qT{ot}g{g}")
                            for ot in range(GH)
                        ],
                        [
                            sb.tile([P, T], mdt, tag=f"kT{ot}", name=f"kT{ot}g{g}")
                            for ot in range(GH)
                        ],
                        [
                            sb.tile([P, NI], mdt, tag=f"v{tt}", bufs=2, name=f"v{tt}g{g}")
                            for tt in range(NKT)
                        ],
                    )

                # ---------------- projection generators ----------------
                def proj_qk_gen(g, ot, copy_eng=None):
                    wq_g, wk_g, _ = W[g]
                    qT_g, kT_g, _ = QKV[g]
                    for wlist, dst in ((wq_g, qT_g[ot]), (wk_g, kT_g[ot])):
                        for tci in range(2):
                            pp = ps.tile([P, NI], f32, tag="pp", bufs=2, name="pp")
                            for ki, kt in enumerate(KT_ORDER):
                                nc.tensor.matmul(
                                    pp[:],
                                    wlist[kt][:, P * ot : P * (ot + 1)],
                                    xT[kt][:, NI * tci : NI * (tci + 1)],
                                    start=(ki == 0),
                                    stop=(ki == NKT - 1),
                                )
                            psum_to_sbuf(
                                dst[:, NI * tci : NI * (tci + 1)], pp[:], copy_eng
                            )
                            yield

                def proj_v_gen(g):
                    _, _, wv_g = W[g]
                    _, _, v_g = QKV[g]
                    for tt in range(NKT):
                        pp = ps.tile([P, NI], f32, tag="pp", bufs=2, name="pp")
                        for ki, kt in enumerate(KT_ORDER):
                            nc.tensor.matmul(
                                pp[:],
                                xT[kt][:, P * tt : P * (tt + 1)],
                                wv_g[kt][:],
                                start=(ki == 0),
                                stop=(ki == NKT - 1),
                            )
                        psum_to_sbuf(v_g[tt][:], pp[:])
                        yield

                def wload_gen(g):
                    load_weights(g)
                    return
                    yield  # noqa: unreachable - makes this a generator

                # ---------------- attention generator ----------------
                def attn_gen(g, ot):
                    h = GH * g + ot
                    last_unit = g == NG - 1 and ot == GH - 1
                    qT_g, kT_g, v_g = QKV[g]
                    qh, kh = qT_g[ot], kT_g[ot]

                    # HW rejects fp32r matmuls with tiny output free size
                    # (s3d3_mm_fp32r_restrictions); run those as plain fp32.
                    def smallmm(ap):
                        return ap.bitcast(f32) if use_f32r else ap

                    # last unit runs ic=1 first: ic=0 has no exception path,
                    # so the end-of-program dependency tail is shorter
                    ic_order = (1, 0) if last_unit else (0, 1)
                    for ic in ic_order:
                        jts = list(range(8)) if ic == 0 else [4, 5, 6, 7]
                        nj = len(jts)

                        u_ps = ps.tile([P, NI], f32, tag="u", bufs=2, name="u_ps")
                        c_ps = ps.tile([P, NI], f32, tag="c", bufs=1, name="c_ps")

                        col_ps = colE = None
                        if ic == 1:
                            # raw scores for column i=1023, rows j in [0,512)
                            col_ps = ps.tile([P, 8], f32, tag="col", bufs=1, name="col_ps")
                            for jc in range(4):
                                nc.tensor.matmul(
                                    col_ps[:, jc : jc + 1],
                                    smallmm(kh[:, P * jc : P * (jc + 1)]),
                                    smallmm(qh[:, T - 1 : T]),
                                    start=True,
                                    stop=True,
                                )
                            colE = sb.tile([P, 8], mdt, tag="colE", bufs=2, name="colE")
                            nc.scalar.activation(colE[:, 0:4], col_ps[:, 0:4], Exp)

                        pend = []
                        eacc = [None]

                        def drain_one():
                            idx, jt, e_sb = pend.pop(0)
                            first, last = idx == 0, idx == nj - 1
                            nc.tensor.matmul(
                                u_ps[:],
                                v_g[jt][:, P * ot : P * (ot + 1)],
                                e_sb[:],
                                start=first,
                                stop=last,
                            )
                            # colsum via elementwise tile accumulation (DVE);
                            # one ones-matmul at the end reduces partitions.
                            # SBUF-only chain -> Pool (gpsimd can't touch PSUM)
                            if idx == 0:
                                eacc[0] = e_sb
                            elif idx == 1:
                                acc = sb.tile(
                                    [P, NI], mdt, tag="eacc", bufs=2, name="eacc"
                                )
                                nc.gpsimd.tensor_tensor(
                                    acc[:], eacc[0][:], e_sb[:], ADD
                                )
                                eacc[0] = acc
                            else:
                                nc.gpsimd.tensor_tensor(
                                    eacc[0][:], eacc[0][:], e_sb[:], ADD
                                )

                        for idx, jt in enumerate(jts):
                            # last unit has no proj filler: borrow the idle pp
                            # psum banks to deepen the S pipeline
                            stag = "pp" if (last_unit and idx % 2 == 1) else "s"
                            s_ps = ps.tile([P, NI], f32, tag=stag, bufs=2, name="s_ps")
                            nc.tensor.matmul(
                                s_ps[:],
                                kh[:, P * jt : P * (jt + 1)],
                                qh[:, NI * ic : NI * (ic + 1)],
                                start=True,
                                stop=True,
                            )
                            crossing = (ic == 0 and jt < 4) or (ic == 1 and jt >= 4)
                            if crossing:
                                nc.vector.tensor_tensor(
                                    s_ps[:], s_ps[:], ad[jt][:], ADD
                                )
                            e_sb = sb.tile([P, NI], mdt, tag="e", bufs=6, name="e_sb")
                            nc.scalar.activation(e_sb[:], s_ps[:], Exp)
                            pend.append((idx, jt, e_sb))
                            while len(pend) > 3:
                                drain_one()
                            yield
                        while pend:
                            drain_one()
                        nc.tensor.matmul(
                            c_ps[:], ones[:], eacc[0][:], start=True, stop=True
                        )

                        if ic == 1:
                            # fold the j<512 contributions of column i=1023 in
                            for jc in range(4):
                                nc.tensor.matmul(
                                    col_ps[:, 4:5],
                                    smallmm(v_g[jc][:, P * ot : P * (ot + 1)]),
                                    smallmm(colE[:, jc : jc + 1]),
                                    start=(jc == 0),
                                    stop=(jc == 3),
                                )
                            for jc in range(4):
                                nc.tensor.matmul(
                                    col_ps[:, 5:6],
                                    smallmm(ones[:]),
                                    smallmm(colE[:, jc : jc + 1]),
                                    start=(jc == 0),
                                    stop=(jc == 3),
                                )
                            colsb = sb.tile([P, 2], f32, tag="colsb", bufs=2, name="colsb")
                            nc.scalar.activation(colsb[:], col_ps[:, 4:6], Copy)
                            nc.vector.tensor_tensor(
                                u_ps[:, NI - 1 : NI], u_ps[:, NI - 1 : NI], colsb[:, 0:1], ADD
                            )
                            nc.vector.tensor_tensor(
                                c_ps[:, NI - 1 : NI], c_ps[:, NI - 1 : NI], colsb[:, 1:2], ADD
                            )

                        recip = sb.tile([P, NI], f32, tag="recip", bufs=2, name="recip")
                        o_sb = sb.tile([P, NI], f32, tag="o", bufs=3, name="o_sb")
                        if last_unit and ic == 0:
                            # final epilogue is fully exposed: halve the DVE
                            # chain so the first out-DMA overlaps the second
                            hn = NI // 2
                            for hf in range(2):
                                sl = slice(hn * hf, hn * (hf + 1))
                                nc.vector.reciprocal(recip[:, sl], c_ps[:, sl])
                                nc.vector.tensor_tensor(
                                    o_sb[:, sl], u_ps[:, sl], recip[:, sl], MUL
                                )
                                nc.sync.dma_start(
                                    out_d.ap()[
                                        P * h : P * (h + 1),
                                        NI * ic + hn * hf : NI * ic + hn * (hf + 1),
                                    ],
                                    o_sb[:, sl],
                                )
                        else:
                            nc.vector.reciprocal(recip[:], c_ps[:])
                            nc.vector.tensor_tensor(o_sb[:], u_ps[:], recip[:], MUL)
                            nc.sync.dma_start(
                                out_d.ap()[P * h : P * (h + 1), NI * ic : NI * (ic + 1)],
                                o_sb[:],
                            )
                        yield

                # ---------------- schedule ----------------
                load_weights(0)
                ones = sb.tile([P, P], mdt, tag="ones", name="ones")
                nc.gpsimd.dma_start(ones[:], on_d.ap()[:])
                for jt in range(4, 8):  # ic=1 crossing blocks, needed later
                    t = sb.tile([P, NI], f32, tag=f"ad{jt}", name=f"ad{jt}")
                    nc.gpsimd.dma_start(t[:], ad_d.ap()[:, NI * jt : NI * (jt + 1)])
                    ad[jt] = t
                for _ in proj_qk_gen(0, 0, copy_eng=nc.vector):
                    pass

                # Filler generators are window-scoped: proj work for (g1, ot)
                # may only be emitted strictly after attn(g0, ot) has finished
                # emitting (WAR hazards on the single-buffered qT/kT/w tiles
                # would otherwise deadlock the in-order engine queues).
                windows = {
                    (0, 0): [proj_v_gen(0), proj_qk_gen(0, 1)],
                    (0, 1): [proj_qk_gen(0, 2)],
                    (0, 2): [proj_qk_gen(0, 3), wload_gen(1), proj_qk_gen(1, 0)],
                    (0, 3): [proj_v_gen(1)],
                    (1, 0): [proj_qk_gen(1, 1)],
                    (1, 1): [proj_qk_gen(1, 2)],
                    (1, 2): [proj_qk_gen(1, 3)],
                }

                for g in range(NG):
                    for ot in range(GH):
                        filler = deque(windows.get((g, ot), []))

                        def pump(n):
                            while n > 0 and filler:
                                try:
                                    next(filler[0])
                                    n -= 1
                                except StopIteration:
                                    filler.popleft()

                        for _ in attn_gen(g, ot):
                            pump(1)
                        pump(10**9)  # drain before the next unit starts

            for _rep in range(reps):
                emit()

    if compile:
        nc.compile()
    return nc


def _get_program():
    global _PROGRAM
    if _PROGRAM is None:
        _PROGRAM = build_program()
    return _PROGRAM


def make_in_maps(x, Wq, Wk, Wv):
    scale = 1.0 / np.sqrt(np.float32(DH))
    wqT = np.ascontiguousarray(np.asarray(Wq, np.float32).T * scale)
    wkT = np.ascontiguousarray(np.asarray(Wk, np.float32).T)
    wvT = np.ascontiguousarray(np.asarray(Wv, np.float32).T)
    adders = _adder_patterns()
    ones = np.ones((P, P), np.float32)
    x = np.asarray(x, np.float32)
    if MM_DT == "bf16":
        import ml_dtypes

        bf16 = ml_dtypes.bfloat16
        wqT, wkT, wvT = wqT.astype(bf16), wkT.astype(bf16), wvT.astype(bf16)
        ones = ones.astype(bf16)
        x = x.astype(bf16)
    in_maps = []
    for b in range(B):
        in_maps.append(
            {
                "xT": np.ascontiguousarray(x[b].T),
                "wqT": wqT,
                "wkT": wkT,
                "wvT": wvT,
                "adders": adders,
                "ones_t": ones,
            }
        )
    return in_maps


def kernel(x, mask, Wq, Wk, Wv, _trace=False):
    from concourse.bass_utils import run_bass_kernel_spmd

    nc = _get_program()
    in_maps = make_in_maps(x, Wq, Wk, Wv)
    res = run_bass_kernel_spmd(nc, in_maps, core_ids=list(range(B)), trace=_trace)
    out = np.stack([res.results[b]["out"] for b in range(B)], axis=0)
    out = np.swapaxes(out, 1, 2)  # device stores out.T
    out = out * np.asarray(mask, np.float32)[:, :, None]
    out = np.ascontiguousarray(out, np.float32)
    if _trace:
        kernel.last_results = res
    return out



# revision 20
# speedup vs baseline: 301.0931x; 1.0662x over previous
"""Trainium2 Bass kernel for causal ("FORWARD" direction) multi-head attention.

Reference computation (per batch b, n_heads=8, d=128):
  Q = x @ Wq.T ; K = x @ Wk.T ; V = x @ Wv.T          (nn.Linear, no bias)
  scores[h,i,j] = (Qh[i] . Kh[j]) / sqrt(d)
  scores += -10000 where j <= i   (keeps strict upper triangle j > i)
  attn = softmax(scores, axis=j) ; out = attn @ Vh ; concat heads
  Row i=1023 is fully masked; jax softmax's max-subtraction makes it equal
  softmax of the *raw* scores, so the kernel keeps column i=1023 unmasked.

Sharding: data-parallel over batch B=8 -> 8 cores, no collectives.

Device layout (per core, everything transposed so the softmax reduction is a
matmul-friendly partition-dim reduction):
  xT[k,t]       : x.T                                  [1024,1024]
  qT/kT[o,t]    : per head-group of 4 heads            via Wq.T/Wk.T as lhsT
  v[t,o]        : natural V                            via xT as lhsT
  S_T[j,i]      = kT_tile.T @ qT  (contraction over d=128, single tile)
  expS          = exp(S_T + adder)   (adder patterns precomputed on host)
  U_T[dd,i]     = sum_j V[j,dd] expS[j,i]   (matmul accum over j tiles)
  colsum[*,i]   = ones.T @ expS             (partition-broadcast row of sums)
  out_T         = U_T * reciprocal(colsum) -> DRAM (host transposes back)

Scheduling: projections for the NEXT head-group are emitted interleaved with
attention of the current group, so PE fills the gaps where it would otherwise
wait on Activation (exp) results.

Performance notes (HW-measured via amortized reps-in-NEFF slope):
  - This hardware streams matmul moving operands at 1 column/cycle @
    ~1.28 GHz for f32r AND bf16 alike (no warm 2.4 GHz state, no 2x bf16
    column rate).  Measured: [128,128]@[128,512] MM = 417.6 ns = pure
    512-column streaming; N=256 -> 259 ns, N=128 -> 287 ns (large fixed
    per-MM overhead below N=512).
  - The all-N=512 checkpoint (592 MMs = 303K moving columns) measured
    237.2 us = its column-streaming floor (zero PE bubbles).  SPLITTING
    live blocks finer (N=256/128) loses to the per-MM overhead, but
    NARROWING each crossing tile's moving width to its live prefix cuts
    dead columns at unchanged MM count: ic=0 tile jt is live only for
    i < 128*(jt+1); rotating the ic=1 window (column 1023 stored first,
    un-rotated host-side by unpermute_out) makes ic=1 tiles live-prefix
    contiguous too.  Measured: 237.2 -> 214.2 us (narrow ic=0 + folding
    the 4 csum-fold matmuls into 1 via a DVE reduce) -> ~205 us expected
    with ic=1 narrowing.  fp8 (DoubleRow) loses to numerics (3.6%
    per-element noise does not average out in random-sign sums -> ~5%
    output error vs the 2e-2 gate); bf16 gains nothing (same column
    rate) and adds error, hence f32r.
"""

import os
import sys
from collections import deque

import numpy as np

if "/opt/trn_rl_repo" not in sys.path:
    sys.path.insert(0, "/opt/trn_rl_repo")

B, T, D, H, DH = 8, 1024, 1024, 8, 128
P = 128          # partition tile
NI = 512         # i-chunk (moving free size)
NG, GH = 2, 4    # head groups x heads per group
NKT = T // P     # 8 contraction tiles
# f32r (fp32 bits, full-rate matmul streaming) is the default: measured on
# this hardware, bf16 matmuls stream at the same 1 column/cycle rate as
# f32r (no 2x), so bf16 would only add quantization error.
MM_DT = os.environ.get("KERNEL_MM_DT", "f32r")  # f32r | f32 | bf16

_PROGRAM = None  # cached compiled Bass program


def _adder_patterns() -> np.ndarray:
    """[128, 8*512] f32. Blocks 0..3: adders for tiles (jt, ic=0); blocks
    4..7: adders for tiles (jt, ic=1) in the ROTATED ic=1 window order
    (position 0 = column i=1023 kept raw, positions 1..511 = columns
    512..1022).  The rotation puts the exception column inside every
    narrowed tile's live prefix."""
    ad = np.zeros((P, 8, NI), np.float32)
    j = np.arange(P)
    i = np.arange(NI)
    for jt in range(4):
        ad[:, jt, :] = np.where((P * jt + j)[:, None] <= i[None, :], -10000.0, 0.0)
    gi = np.concatenate([[T - 1], NI + np.arange(NI - 1)])  # window pos -> i
    for jt in range(4, 8):
        blk = np.where((P * jt + j)[:, None] <= gi[None, :], -10000.0, 0.0)
        blk[:, 0] = 0.0  # column i=1023 stays raw
        ad[:, jt, :] = blk
    return np.ascontiguousarray(ad.reshape(P, 8 * NI))


def build_program(mm_dt: str = MM_DT, compile: bool = True, reps: int = 1):
    import concourse.bass as bass  # noqa: F401
    import concourse.tile as tile
    from concourse import bacc, mybir

    f32 = mybir.dt.float32
    use_f32r = mm_dt == "f32r"
    mdt = {
        "bf16": mybir.dt.bfloat16,
        "f32r": mybir.dt.float32r,
        "f32": mybir.dt.float32,
    }[mm_dt]
    Exp = mybir.ActivationFunctionType.Exp
    Copy = mybir.ActivationFunctionType.Copy
    ADD = mybir.AluOpType.add
    MUL = mybir.AluOpType.mult

    nc = bacc.Bacc(
        "TRN2",
        target_bir_lowering=False,
        debug=False,
        enable_asserts=False,
        num_devices=B,
    )

    xT_d = nc.dram_tensor("xT", [D, T], mdt, kind="ExternalInput")
    wq_d = nc.dram_tensor("wqT", [D, D], mdt, kind="ExternalInput")
    wk_d = nc.dram_tensor("wkT", [D, D], mdt, kind="ExternalInput")
    wv_d = nc.dram_tensor("wvT", [D, D], mdt, kind="ExternalInput")
    ad_d = nc.dram_tensor("adders", [P, 8 * NI], f32, kind="ExternalInput")
    on_d = nc.dram_tensor("ones_t", [P, P], mdt, kind="ExternalInput")
    # out is stored TRANSPOSED ([D, T]); the host wrapper transposes back.
    out_d = nc.dram_tensor("out", [D, T], f32, kind="ExternalOutput")

    with tile.TileContext(nc) as tc:
        with (
            tc.tile_pool(name="sb", bufs=1) as sb,
            tc.tile_pool(name="ps", bufs=1, space="PSUM") as ps,
        ):
            KT_ORDER = (4, 0, 5, 1, 6, 2, 7, 3)

            def emit():
                # ---------------- resident loads ----------------
                # xT split across the SP and ACT DMA queues; weights on
                # Pool/SP; adder blocks trickle in on Pool in first-use
                # order.  kt loops consume in KT_ORDER = arrival order.
                xT = [None] * NKT

                def load_xT(k, eng):
                    t = sb.tile([P, T], mdt, tag=f"xT{k}", name=f"xT{k}")
                    eng.dma_start(t[:], xT_d.ap()[P * k : P * (k + 1), :])
                    xT[k] = t

                for k in (4, 0, 5, 1):
                    load_xT(k, nc.sync)
                for k in (6, 2, 7, 3):
                    load_xT(k, nc.scalar)
                ad = [None] * 8
                for jt in range(4):  # ic=0 crossing blocks, needed first
                    t = sb.tile([P, NI], f32, tag=f"ad{jt}", name=f"ad{jt}")
                    nc.scalar.dma_start(t[:], ad_d.ap()[:, NI * jt : NI * (jt + 1)])
                    ad[jt] = t

                copy_flip = [0]

                def psum_to_sbuf(dst_ap, src_ap, eng=None):
                    if eng is None:
                        eng = nc.scalar if copy_flip[0] % 2 == 0 else nc.vector
                        copy_flip[0] += 1
                    if eng is nc.scalar:
                        eng.activation(dst_ap, src_ap, Copy)
                    else:
                        eng.tensor_copy(dst_ap, src_ap)


                W, QKV = {}, {}

                def load_weights(g):
                    dram = {"wq": wq_d, "wk": wk_d, "wv": wv_d}
                    if g == 0:
                        placement = {
                            "wq": [(kt, nc.gpsimd) for kt in KT_ORDER],
                            "wk": [(4, nc.sync), (0, nc.sync), (5, nc.sync),
                                   (1, nc.sync), (6, nc.gpsimd), (2, nc.gpsimd),
                                   (7, nc.gpsimd), (3, nc.gpsimd)],
                            "wv": [(4, nc.sync), (0, nc.sync), (5, nc.sync),
                                   (1, nc.sync), (6, nc.gpsimd), (2, nc.gpsimd),
                                   (7, nc.gpsimd), (3, nc.gpsimd)],
                        }
                        order = ("wq", "wk", "wv")
                    else:
                        placement = {
                            nm: [(kt, nc.sync) for kt in KT_ORDER]
                            for nm in ("wv", "wq", "wk")
                        }
                        order = ("wv", "wq", "wk")
                    lists = {}
                    for nm in order:
                        lst = [None] * NKT
                        for kt, eng in placement[nm]:
                            w = sb.tile(
                                [P, NI], mdt, tag=f"{nm}{kt}", name=f"{nm}{kt}g{g}"
                            )
                            eng.dma_start(
                                w[:],
                                dram[nm].ap()[P * kt : P * (kt + 1), NI * g : NI * (g + 1)],
                            )
                            lst[kt] = w
                        lists[nm] = lst
                    W[g] = (lists["wq"], lists["wk"], lists["wv"])
                    QKV[g] = (
                        [
                            sb.tile([P, T], mdt, tag=f"qT{ot}", name=f"qT{ot}g{g}")
                            for ot in range(GH)
                        ],
                        [
                            sb.tile([P, T], mdt, tag=f"kT{ot}", name=f"kT{ot}g{g}")
                            for ot in range(GH)
                        ],
                        [
                            sb.tile([P, NI], mdt, tag=f"v{tt}", bufs=2, name=f"v{tt}g{g}")
                            for tt in range(NKT)
                        ],
                    )

                # ---------------- projection generators ----------------
                def proj_qk_gen(g, ot, copy_eng=None):
                    wq_g, wk_g, _ = W[g]
                    qT_g, kT_g, _ = QKV[g]
                    for wlist, dst in ((wq_g, qT_g[ot]), (wk_g, kT_g[ot])):
                        is_q = wlist is wq_g
                        for tci in range(2):
                            pp = ps.tile([P, NI], f32, tag="pp", bufs=2, name="pp")
                            for ki, kt in enumerate(KT_ORDER):
                                nc.tensor.matmul(
                                    pp[:],
                                    wlist[kt][:, P * ot : P * (ot + 1)],
                                    xT[kt][:, NI * tci : NI * (tci + 1)],
                                    start=(ki == 0),
                                    stop=(ki == NKT - 1),
                                )
                            if is_q and tci == 1:
                                # store qT's second half ROTATED by one: column
                                # 1023 lands at position 512 so the ic=1 window
                                # reads [1023, 512, 513, ...] contiguously.
                                psum_to_sbuf(
                                    dst[:, NI + 1 : T], pp[:, 0 : NI - 1], copy_eng
                                )
                                psum_to_sbuf(
                                    dst[:, NI : NI + 1], pp[:, NI - 1 : NI], copy_eng
                                )
                            else:
                                psum_to_sbuf(
                                    dst[:, NI * tci : NI * (tci + 1)], pp[:], copy_eng
                                )
                            yield

                def proj_v_gen(g):
                    _, _, wv_g = W[g]
                    _, _, v_g = QKV[g]
                    for tt in range(NKT):
                        pp = ps.tile([P, NI], f32, tag="pp", bufs=2, name="pp")
                        for ki, kt in enumerate(KT_ORDER):
                            nc.tensor.matmul(
                                pp[:],
                                xT[kt][:, P * tt : P * (tt + 1)],
                                wv_g[kt][:],
                                start=(ki == 0),
                                stop=(ki == NKT - 1),
                            )
                        psum_to_sbuf(v_g[tt][:], pp[:])
                        yield

                def wload_gen(g):
                    load_weights(g)
                    return
                    yield  # noqa: unreachable - makes this a generator

                # ---------------- attention generator ----------------
                def attn_gen(g, ot):
                    h = GH * g + ot
                    last_unit = g == NG - 1 and ot == GH - 1
                    qT_g, kT_g, v_g = QKV[g]
                    qh, kh = qT_g[ot], kT_g[ot]

                    # HW rejects fp32r matmuls with tiny output free size
                    # (s3d3_mm_fp32r_restrictions); run those as plain fp32.
                    def smallmm(ap):
                        return ap.bitcast(f32) if use_f32r else ap

                    # last unit runs ic=1 first: ic=0 has no exception path,
                    # so the end-of-program dependency tail is shorter
                    ic_order = (1, 0) if last_unit else (0, 1)
                    for ic in ic_order:
                        # Full-width tiles drain first so the start AV matmul
                        # covers the whole PSUM bank (narrowed tiles then
                        # pure-accumulate into already-written regions); for
                        # unit (0,0) the ic=0 order also keeps jts[k] <= k+3+1
                        # = the proj_v filler's progress at that drain.
                        jts = [3, 4, 5, 6, 0, 1, 2, 7] if ic == 0 else [7, 4, 5, 6]
                        nj = len(jts)

                        u_ps = ps.tile([P, NI], f32, tag="u", bufs=2, name="u_ps")
                        c_ps = ps.tile([P, NI], f32, tag="c", bufs=1, name="c_ps")

                        col_ps = colE = None
                        if ic == 1:
                            # raw scores for column i=1023, rows j in [0,512);
                            # that column is stored at position NI (rotated)
                            col_ps = ps.tile([P, 8], f32, tag="col", bufs=1, name="col_ps")
                            for jc in range(4):
                                nc.tensor.matmul(
                                    col_ps[:, jc : jc + 1],
                                    smallmm(kh[:, P * jc : P * (jc + 1)]),
                                    smallmm(qh[:, NI : NI + 1]),
                                    start=True,
                                    stop=True,
                                )
                            colE = sb.tile([P, 8], mdt, tag="colE", bufs=2, name="colE")
                            nc.scalar.activation(colE[:, 0:4], col_ps[:, 0:4], Exp)

                        pend = []
                        eacc = [None]

                        def drain_one():
                            idx, jt, w, e_sb = pend.pop(0)
                            first, last = idx == 0, idx == nj - 1
                            nc.tensor.matmul(
                                u_ps[:, :w],
                                v_g[jt][:, P * ot : P * (ot + 1)],
                                e_sb[:, :w],
                                start=first,
                                stop=last,
                            )
                            # colsum via elementwise tile accumulation (Pool);
                            # one ones-matmul at the end reduces partitions.
                            # SBUF-only chain -> Pool (gpsimd can't touch PSUM).
                            # jts ordering guarantees idx 0 and 1 are full-width
                            # tiles, so the alias/alloc chain stays exact; the
                            # narrowed tiles accumulate in place on their slice.
                            if idx == 0:
                                eacc[0] = e_sb
                            elif idx == 1:
                                acc = sb.tile(
                                    [P, NI], mdt, tag="eacc", bufs=2, name="eacc"
                                )
                                if w == NI:
                                    nc.gpsimd.tensor_tensor(
                                        acc[:], eacc[0][:], e_sb[:], ADD
                                    )
                                else:
                                    nc.gpsimd.tensor_tensor(
                                        acc[:, :w], eacc[0][:, :w], e_sb[:, :w], ADD
                                    )
                                    nc.gpsimd.tensor_copy(
                                        acc[:, w:], eacc[0][:, w:]
                                    )
                                eacc[0] = acc
                            else:
                                nc.gpsimd.tensor_tensor(
                                    eacc[0][:, :w], eacc[0][:, :w], e_sb[:, :w], ADD
                                )

                        for idx, jt in enumerate(jts):
                            # Crossing tiles are dead beyond their diagonal:
                            # ic=0 tile jt is live only for i < 128*(jt+1);
                            # ic=1 tile jt (rotated window) only for positions
                            # < 128*(jt-3) (incl. pos 0 = raw column 1023).
                            # Narrow the moving width so S/exp/AV skip the
                            # dead columns.
                            if ic == 0:
                                w = min(P * (jt + 1), NI)
                            else:
                                w = NI if jt == 7 else P * (jt - 3)
                            # last unit has no proj filler: borrow the idle pp
                            # psum banks to deepen the S pipeline
                            stag = "pp" if (last_unit and idx % 2 == 1) else "s"
                            s_ps = ps.tile([P, NI], f32, tag=stag, bufs=2, name="s_ps")
                            nc.tensor.matmul(
                                s_ps[:, :w],
                                kh[:, P * jt : P * (jt + 1)],
                                qh[:, NI * ic : NI * ic + w],
                                start=True,
                                stop=True,
                            )
                            crossing = (ic == 0 and jt < 4) or (ic == 1 and jt >= 4)
                            if crossing:
                                nc.vector.tensor_tensor(
                                    s_ps[:, :w], s_ps[:, :w], ad[jt][:, :w], ADD
                                )
                            e_sb = sb.tile([P, NI], mdt, tag="e", bufs=6, name="e_sb")
                            nc.scalar.activation(e_sb[:, :w], s_ps[:, :w], Exp)
                            pend.append((idx, jt, w, e_sb))
                            # unit (0,0) fills v_g concurrently (proj_v filler):
                            # one extra pipeline step so v[jts[0]]=v3 is emitted
                            # before the first AV drain reads it.
                            lag = 4 if (g == 0 and ot == 0 and ic == 0) else 3
                            while len(pend) > lag:
                                drain_one()
                            yield
                        while pend:
                            drain_one()
                        nc.tensor.matmul(
                            c_ps[:], ones[:], eacc[0][:], start=True, stop=True
                        )

                        if ic == 1:
                            # fold the j<512 contributions of column i=1023 in
                            for jc in range(4):
                                nc.tensor.matmul(
                                    col_ps[:, 4:5],
                                    smallmm(v_g[jc][:, P * ot : P * (ot + 1)]),
                                    smallmm(colE[:, jc : jc + 1]),
                                    start=(jc == 0),
                                    stop=(jc == 3),
                                )
                            # sum the 4 per-tile exception exps on DVE (free-dim
                            # reduce), then one 1-col ones-matmul broadcasts the
                            # partition sum — replaces 4 tiny matmuls with 1.
                            colR = sb.tile(
                                [P, 1],
                                f32 if use_f32r else mdt,
                                tag="colR",
                                bufs=2,
                                name="colR",
                            )
                            nc.vector.reduce_sum(
                                colR[:],
                                smallmm(colE[:, 0:4]),
                                axis=mybir.AxisListType.X,
                            )
                            nc.tensor.matmul(
                                col_ps[:, 5:6],
                                smallmm(ones[:]),
                                colR[:],
                                start=True,
                                stop=True,
                            )
                            colsb = sb.tile([P, 2], f32, tag="colsb", bufs=2, name="colsb")
                            nc.scalar.activation(colsb[:], col_ps[:, 4:6], Copy)
                            # column 1023 sits at window position 0 (rotated)
                            nc.vector.tensor_tensor(
                                u_ps[:, 0:1], u_ps[:, 0:1], colsb[:, 0:1], ADD
                            )
                            nc.vector.tensor_tensor(
                                c_ps[:, 0:1], c_ps[:, 0:1], colsb[:, 1:2], ADD
                            )

                        recip = sb.tile([P, NI], f32, tag="recip", bufs=2, name="recip")
                        o_sb = sb.tile([P, NI], f32, tag="o", bufs=3, name="o_sb")
                        if last_unit and ic == 0:
                            # final epilogue is fully exposed: halve the DVE
                            # chain so the first out-DMA overlaps the second
                            hn = NI // 2
                            for hf in range(2):
                                sl = slice(hn * hf, hn * (hf + 1))
                                nc.vector.reciprocal(recip[:, sl], c_ps[:, sl])
                                nc.vector.tensor_tensor(
                                    o_sb[:, sl], u_ps[:, sl], recip[:, sl], MUL
                                )
                                nc.sync.dma_start(
                                    out_d.ap()[
                                        P * h : P * (h + 1),
                                        NI * ic + hn * hf : NI * ic + hn * (hf + 1),
                                    ],
                                    o_sb[:, sl],
                                )
                        else:
                            nc.vector.reciprocal(recip[:], c_ps[:])
                            nc.vector.tensor_tensor(o_sb[:], u_ps[:], recip[:], MUL)
                            nc.sync.dma_start(
                                out_d.ap()[P * h : P * (h + 1), NI * ic : NI * (ic + 1)],
                                o_sb[:],
                            )
                        yield

                # ---------------- schedule ----------------
                load_weights(0)
                ones = sb.tile([P, P], mdt, tag="ones", name="ones")
                nc.gpsimd.dma_start(ones[:], on_d.ap()[:])
                for jt in range(4, 8):  # ic=1 crossing blocks, needed later
                    t = sb.tile([P, NI], f32, tag=f"ad{jt}", name=f"ad{jt}")
                    nc.gpsimd.dma_start(t[:], ad_d.ap()[:, NI * jt : NI * (jt + 1)])
                    ad[jt] = t
                for _ in proj_qk_gen(0, 0, copy_eng=nc.vector):
                    pass

                # Filler generators are window-scoped: proj work for (g1, ot)
                # may only be emitted strictly after attn(g0, ot) has finished
                # emitting (WAR hazards on the single-buffered qT/kT/w tiles
                # would otherwise deadlock the in-order engine queues).
                windows = {
                    (0, 0): [proj_v_gen(0), proj_qk_gen(0, 1)],
                    (0, 1): [proj_qk_gen(0, 2)],
                    (0, 2): [proj_qk_gen(0, 3), wload_gen(1), proj_qk_gen(1, 0)],
                    (0, 3): [proj_v_gen(1)],
                    (1, 0): [proj_qk_gen(1, 1)],
                    (1, 1): [proj_qk_gen(1, 2)],
                    (1, 2): [proj_qk_gen(1, 3)],
                }

                for g in range(NG):
                    for ot in range(GH):
                        filler = deque(windows.get((g, ot), []))

                        def pump(n):
                            while n > 0 and filler:
                                try:
                                    next(filler[0])
                                    n -= 1
                                except StopIteration:
                                    filler.popleft()

                        for _ in attn_gen(g, ot):
                            pump(1)
                        pump(10**9)  # drain before the next unit starts

            for _rep in range(reps):
                emit()

    if compile:
        nc.compile()
    return nc


def _get_program():
    global _PROGRAM
    if _PROGRAM is None:
        _PROGRAM = build_program()
    return _PROGRAM


def make_in_maps(x, Wq, Wk, Wv):
    scale = 1.0 / np.sqrt(np.float32(DH))
    wqT = np.ascontiguousarray(np.asarray(Wq, np.float32).T * scale)
    wkT = np.ascontiguousarray(np.asarray(Wk, np.float32).T)
    wvT = np.ascontiguousarray(np.asarray(Wv, np.float32).T)
    adders = _adder_patterns()
    ones = np.ones((P, P), np.float32)
    x = np.asarray(x, np.float32)
    if MM_DT == "bf16":
        import ml_dtypes

        bf16 = ml_dtypes.bfloat16
        wqT, wkT, wvT = wqT.astype(bf16), wkT.astype(bf16), wvT.astype(bf16)
        ones = ones.astype(bf16)
        x = x.astype(bf16)
    in_maps = []
    for b in range(B):
        in_maps.append(
            {
                "xT": np.ascontiguousarray(x[b].T),
                "wqT": wqT,
                "wkT": wkT,
                "wvT": wvT,
                "adders": adders,
                "ones_t": ones,
            }
        )
    return in_maps


def unpermute_out(out_btd: np.ndarray) -> np.ndarray:
    """Undo the device's rotated ic=1 window: token positions [512..1024) are
    stored as [1023, 512, 513, .., 1022]; roll them back to natural order."""
    out_btd[:, NI:, :] = np.roll(out_btd[:, NI:, :], -1, axis=1)
    return out_btd


def kernel(x, mask, Wq, Wk, Wv, _trace=False):
    from concourse.bass_utils import run_bass_kernel_spmd

    nc = _get_program()
    in_maps = make_in_maps(x, Wq, Wk, Wv)
    res = run_bass_kernel_spmd(nc, in_maps, core_ids=list(range(B)), trace=_trace)
    out = np.stack([res.results[b]["out"] for b in range(B)], axis=0)
    out = np.swapaxes(out, 1, 2)  # device stores out.T
    out = unpermute_out(np.ascontiguousarray(out))
    out = out * np.asarray(mask, np.float32)[:, :, None]
    out = np.ascontiguousarray(out, np.float32)
    if _trace:
        kernel.last_results = res
    return out



# revision 24
# speedup vs baseline: 309.6807x; 1.0285x over previous
"""Trainium2 Bass kernel for causal ("FORWARD" direction) multi-head attention.

Reference computation (per batch b, n_heads=8, d=128):
  Q = x @ Wq.T ; K = x @ Wk.T ; V = x @ Wv.T          (nn.Linear, no bias)
  scores[h,i,j] = (Qh[i] . Kh[j]) / sqrt(d)
  scores += -10000 where j <= i   (keeps strict upper triangle j > i)
  attn = softmax(scores, axis=j) ; out = attn @ Vh ; concat heads
  Row i=1023 is fully masked; jax softmax's max-subtraction makes it equal
  softmax of the *raw* scores, so the kernel keeps column i=1023 unmasked.

Sharding: data-parallel over batch B=8 -> 8 cores, no collectives.

Device layout (per core, everything transposed so the softmax reduction is a
matmul-friendly partition-dim reduction):
  xT[k,t]       : x.T                                  [1024,1024]
  qT/kT[o,t]    : per head-group of 4 heads            via Wq.T/Wk.T as lhsT
  v[t,o]        : natural V                            via xT as lhsT
  S_T[j,i]      = kT_tile.T @ qT  (contraction over d=128, single tile)
  expS          = exp(S_T + adder)   (adder patterns precomputed on host)
  U_T[dd,i]     = sum_j V[j,dd] expS[j,i]   (matmul accum over j tiles)
  colsum[*,i]   = ones.T @ expS             (partition-broadcast row of sums)
  out_T         = U_T * reciprocal(colsum) -> DRAM (host transposes back)

Scheduling: projections for the NEXT head-group are emitted interleaved with
attention of the current group, so PE fills the gaps where it would otherwise
wait on Activation (exp) results.

Performance notes (HW-measured via amortized reps-in-NEFF slope):
  - This hardware streams matmul moving operands at 1 column/cycle @
    ~1.28 GHz for f32r AND bf16 alike (no warm 2.4 GHz state, no 2x bf16
    column rate).  Measured: [128,128]@[128,512] MM = 417.6 ns = pure
    512-column streaming; N=256 -> 259 ns, N=128 -> 287 ns (large fixed
    per-MM overhead below N=512).
  - The all-N=512 checkpoint (592 MMs = 303K moving columns) measured
    237.2 us = its column-streaming floor (zero PE bubbles).  SPLITTING
    live blocks finer (N=256/128) loses to the per-MM overhead, but
    NARROWING each crossing tile's moving width to its live prefix cuts
    dead columns at unchanged MM count: ic=0 tile jt is live only for
    i < 128*(jt+1); rotating the ic=1 window (column 1023 stored first,
    un-rotated host-side by unpermute_out) makes ic=1 tiles live-prefix
    contiguous too.  Measured: 237.2 -> 214.2 us (narrow ic=0 + folding
    the 4 csum-fold matmuls into 1 via a DVE reduce) -> ~205 us expected
    with ic=1 narrowing.  fp8 (DoubleRow) loses to numerics (3.6%
    per-element noise does not average out in random-sign sums -> ~5%
    output error vs the 2e-2 gate); bf16 gains nothing (same column
    rate) and adds error, hence f32r.
"""

import os
import sys
from collections import deque

import numpy as np

if "/opt/trn_rl_repo" not in sys.path:
    sys.path.insert(0, "/opt/trn_rl_repo")

B, T, D, H, DH = 8, 1024, 1024, 8, 128
P = 128          # partition tile
NI = 512         # i-chunk (moving free size)
NG, GH = 2, 4    # head groups x heads per group
NKT = T // P     # 8 contraction tiles
# f32r (fp32 bits, full-rate matmul streaming) is the default: measured on
# this hardware, bf16 matmuls stream at the same 1 column/cycle rate as
# f32r (no 2x), so bf16 would only add quantization error.
MM_DT = os.environ.get("KERNEL_MM_DT", "f32r")  # f32r | f32 | bf16

_PROGRAM = None  # cached compiled Bass program


def _adder_patterns() -> np.ndarray:
    """[128, 8*512] f32. Blocks 0..3: adders for tiles (jt, ic=0); blocks
    4..7: adders for tiles (jt, ic=1) in the ROTATED ic=1 window order
    (position 0 = column i=1023 kept raw, positions 1..511 = columns
    512..1022).  The rotation puts the exception column inside every
    narrowed tile's live prefix."""
    ad = np.zeros((P, 8, NI), np.float32)
    j = np.arange(P)
    i = np.arange(NI)
    for jt in range(4):
        ad[:, jt, :] = np.where((P * jt + j)[:, None] <= i[None, :], -10000.0, 0.0)
    gi = np.concatenate([[T - 1], NI + np.arange(NI - 1)])  # window pos -> i
    for jt in range(4, 8):
        blk = np.where((P * jt + j)[:, None] <= gi[None, :], -10000.0, 0.0)
        blk[:, 0] = 0.0  # column i=1023 stays raw
        ad[:, jt, :] = blk
    return np.ascontiguousarray(ad.reshape(P, 8 * NI))


def build_program(mm_dt: str = MM_DT, compile: bool = True, reps: int = 1):
    import concourse.bass as bass  # noqa: F401
    import concourse.tile as tile
    from concourse import bacc, mybir

    f32 = mybir.dt.float32
    use_f32r = mm_dt == "f32r"
    mdt = {
        "bf16": mybir.dt.bfloat16,
        "f32r": mybir.dt.float32r,
        "f32": mybir.dt.float32,
    }[mm_dt]
    Exp = mybir.ActivationFunctionType.Exp
    Copy = mybir.ActivationFunctionType.Copy
    ADD = mybir.AluOpType.add
    MUL = mybir.AluOpType.mult

    nc = bacc.Bacc(
        "TRN2",
        target_bir_lowering=False,
        debug=False,
        enable_asserts=False,
        num_devices=B,
    )

    xT_d = nc.dram_tensor("xT", [D, T], mdt, kind="ExternalInput")
    wq_d = nc.dram_tensor("wqT", [D, D], mdt, kind="ExternalInput")
    wk_d = nc.dram_tensor("wkT", [D, D], mdt, kind="ExternalInput")
    wv_d = nc.dram_tensor("wvT", [D, D], mdt, kind="ExternalInput")
    ad_d = nc.dram_tensor("adders", [P, 8 * NI], f32, kind="ExternalInput")
    on_d = nc.dram_tensor("ones_t", [P, P], mdt, kind="ExternalInput")
    # out is stored TRANSPOSED ([D, T]); the host wrapper transposes back.
    out_d = nc.dram_tensor("out", [D, T], f32, kind="ExternalOutput")

    with tile.TileContext(nc) as tc:
        with (
            tc.tile_pool(name="sb", bufs=1) as sb,
            tc.tile_pool(name="ps", bufs=1, space="PSUM") as ps,
        ):
            KT_ORDER = (4, 0, 5, 1, 6, 2, 7, 3)

            def emit():
                # ---------------- resident loads ----------------
                # xT split across the SP and ACT DMA queues; weights on
                # Pool/SP; adder blocks trickle in on Pool in first-use
                # order.  kt loops consume in KT_ORDER = arrival order.
                xT = [None] * NKT

                def load_xT(k, eng):
                    t = sb.tile([P, T], mdt, tag=f"xT{k}", name=f"xT{k}")
                    eng.dma_start(t[:], xT_d.ap()[P * k : P * (k + 1), :])
                    xT[k] = t

                for k in (4, 0, 5, 1):
                    load_xT(k, nc.sync)
                for k in (6, 2, 7, 3):
                    load_xT(k, nc.scalar)
                ad = [None] * 8
                for jt in range(4):  # ic=0 crossing blocks, needed first
                    t = sb.tile([P, NI], f32, tag=f"ad{jt}", name=f"ad{jt}")
                    nc.scalar.dma_start(t[:], ad_d.ap()[:, NI * jt : NI * (jt + 1)])
                    ad[jt] = t

                copy_flip = [0]

                def psum_to_sbuf(dst_ap, src_ap, eng=None):
                    if eng is None:
                        eng = nc.scalar if copy_flip[0] % 2 == 0 else nc.vector
                        copy_flip[0] += 1
                    if eng is nc.scalar:
                        eng.activation(dst_ap, src_ap, Copy)
                    else:
                        eng.tensor_copy(dst_ap, src_ap)


                W, QKV = {}, {}

                def load_weights(g):
                    dram = {"wq": wq_d, "wk": wk_d, "wv": wv_d}
                    if g == 0:
                        placement = {
                            "wq": [(kt, nc.gpsimd) for kt in KT_ORDER],
                            "wk": [(4, nc.sync), (0, nc.sync), (5, nc.sync),
                                   (1, nc.sync), (6, nc.gpsimd), (2, nc.gpsimd),
                                   (7, nc.gpsimd), (3, nc.gpsimd)],
                            "wv": [(4, nc.sync), (0, nc.sync), (5, nc.sync),
                                   (1, nc.sync), (6, nc.gpsimd), (2, nc.gpsimd),
                                   (7, nc.gpsimd), (3, nc.gpsimd)],
                        }
                        order = ("wq", "wk", "wv")
                    else:
                        placement = {
                            nm: [(kt, nc.sync) for kt in KT_ORDER]
                            for nm in ("wv", "wq", "wk")
                        }
                        order = ("wv", "wq", "wk")
                    lists = {}
                    for nm in order:
                        lst = [None] * NKT
                        for kt, eng in placement[nm]:
                            w = sb.tile(
                                [P, NI], mdt, tag=f"{nm}{kt}", name=f"{nm}{kt}g{g}"
                            )
                            eng.dma_start(
                                w[:],
                                dram[nm].ap()[P * kt : P * (kt + 1), NI * g : NI * (g + 1)],
                            )
                            lst[kt] = w
                        lists[nm] = lst
                    W[g] = (lists["wq"], lists["wk"], lists["wv"])
                    QKV[g] = (
                        [
                            sb.tile([P, T], mdt, tag=f"qT{ot}", name=f"qT{ot}g{g}")
                            for ot in range(GH)
                        ],
                        [
                            sb.tile([P, T], mdt, tag=f"kT{ot}", name=f"kT{ot}g{g}")
                            for ot in range(GH)
                        ],
                        [
                            sb.tile([P, NI], mdt, tag=f"v{tt}", bufs=2, name=f"v{tt}g{g}")
                            for tt in range(NKT)
                        ],
                    )

                # ---------------- projection generators ----------------
                def proj_qk_gen(g, ot, copy_eng=None):
                    wq_g, wk_g, _ = W[g]
                    qT_g, kT_g, _ = QKV[g]
                    for wlist, dst in ((wq_g, qT_g[ot]), (wk_g, kT_g[ot])):
                        is_q = wlist is wq_g
                        for tci in range(2):
                            pp = ps.tile([P, NI], f32, tag="pp", bufs=2, name="pp")
                            for ki, kt in enumerate(KT_ORDER):
                                nc.tensor.matmul(
                                    pp[:],
                                    wlist[kt][:, P * ot : P * (ot + 1)],
                                    xT[kt][:, NI * tci : NI * (tci + 1)],
                                    start=(ki == 0),
                                    stop=(ki == NKT - 1),
                                )
                            if is_q and tci == 1:
                                # store qT's second half ROTATED by one: column
                                # 1023 lands at position 512 so the ic=1 window
                                # reads [1023, 512, 513, ...] contiguously.
                                psum_to_sbuf(
                                    dst[:, NI + 1 : T], pp[:, 0 : NI - 1], copy_eng
                                )
                                psum_to_sbuf(
                                    dst[:, NI : NI + 1], pp[:, NI - 1 : NI], copy_eng
                                )
                            else:
                                psum_to_sbuf(
                                    dst[:, NI * tci : NI * (tci + 1)], pp[:], copy_eng
                                )
                            yield

                def proj_v_gen(g):
                    _, _, wv_g = W[g]
                    _, _, v_g = QKV[g]
                    for tt in range(NKT):
                        pp = ps.tile([P, NI], f32, tag="pp", bufs=2, name="pp")
                        for ki, kt in enumerate(KT_ORDER):
                            nc.tensor.matmul(
                                pp[:],
                                xT[kt][:, P * tt : P * (tt + 1)],
                                wv_g[kt][:],
                                start=(ki == 0),
                                stop=(ki == NKT - 1),
                            )
                        psum_to_sbuf(v_g[tt][:], pp[:])
                        yield

                def wload_gen(g):
                    load_weights(g)
                    return
                    yield  # noqa: unreachable - makes this a generator

                # ---------------- attention generator ----------------
                def attn_gen(g, ot):
                    h = GH * g + ot
                    last_unit = g == NG - 1 and ot == GH - 1
                    qT_g, kT_g, v_g = QKV[g]
                    qh, kh = qT_g[ot], kT_g[ot]

                    # HW rejects fp32r matmuls with tiny output free size
                    # (s3d3_mm_fp32r_restrictions); run those as plain fp32.
                    def smallmm(ap):
                        return ap.bitcast(f32) if use_f32r else ap

                    # last unit runs ic=1 first: ic=0 has no exception path,
                    # so the end-of-program dependency tail is shorter
                    ic_order = (1, 0) if last_unit else (0, 1)
                    for ic in ic_order:
                        # Full-width tiles drain first so the start AV matmul
                        # covers the whole PSUM bank (narrowed tiles then
                        # pure-accumulate into already-written regions); for
                        # unit (0,0) the ic=0 order also keeps jts[k] <= k+3+1
                        # = the proj_v filler's progress at that drain.
                        jts = [3, 4, 5, 6, 0, 1, 2, 7] if ic == 0 else [7, 4, 5, 6]
                        nj = len(jts)

                        u_ps = ps.tile([P, NI], f32, tag="u", bufs=2, name="u_ps")
                        c_ps = ps.tile([P, NI], f32, tag="c", bufs=1, name="c_ps")

                        col_ps = colE = None
                        if ic == 1:
                            # raw scores for column i=1023, rows j in [0,512);
                            # that column is stored at position NI (rotated)
                            col_ps = ps.tile([P, 8], f32, tag="col", bufs=1, name="col_ps")
                            for jc in range(4):
                                nc.tensor.matmul(
                                    col_ps[:, jc : jc + 1],
                                    smallmm(kh[:, P * jc : P * (jc + 1)]),
                                    smallmm(qh[:, NI : NI + 1]),
                                    start=True,
                                    stop=True,
                                )
                            colE = sb.tile([P, 8], mdt, tag="colE", bufs=2, name="colE")
                            nc.scalar.activation(colE[:, 0:4], col_ps[:, 0:4], Exp)

                        pend = []
                        eacc = [None]

                        def drain_one():
                            idx, jt, w, e_sb = pend.pop(0)
                            first, last = idx == 0, idx == nj - 1
                            nc.tensor.matmul(
                                u_ps[:, :w],
                                v_g[jt][:, P * ot : P * (ot + 1)],
                                e_sb[:, :w],
                                start=first,
                                stop=last,
                            )
                            # colsum via elementwise tile accumulation (Pool);
                            # one ones-matmul at the end reduces partitions.
                            # SBUF-only chain -> Pool (gpsimd can't touch PSUM).
                            # jts ordering guarantees idx 0 and 1 are full-width
                            # tiles, so the alias/alloc chain stays exact; the
                            # narrowed tiles accumulate in place on their slice.
                            if idx == 0:
                                eacc[0] = e_sb
                            elif idx == 1:
                                acc = sb.tile(
                                    [P, NI], mdt, tag="eacc", bufs=2, name="eacc"
                                )
                                if w == NI:
                                    nc.gpsimd.tensor_tensor(
                                        acc[:], eacc[0][:], e_sb[:], ADD
                                    )
                                else:
                                    nc.gpsimd.tensor_tensor(
                                        acc[:, :w], eacc[0][:, :w], e_sb[:, :w], ADD
                                    )
                                    nc.gpsimd.tensor_copy(
                                        acc[:, w:], eacc[0][:, w:]
                                    )
                                eacc[0] = acc
                            else:
                                nc.gpsimd.tensor_tensor(
                                    eacc[0][:, :w], eacc[0][:, :w], e_sb[:, :w], ADD
                                )

                        for idx, jt in enumerate(jts):
                            # Crossing tiles are dead beyond their diagonal:
                            # ic=0 tile jt is live only for i < 128*(jt+1);
                            # ic=1 tile jt (rotated window) only for positions
                            # < 128*(jt-3) (incl. pos 0 = raw column 1023).
                            # Narrow the moving width so S/exp/AV skip the
                            # dead columns.
                            if ic == 0:
                                w = min(P * (jt + 1), NI)
                            else:
                                w = NI if jt == 7 else P * (jt - 3)
                            # last unit has no proj filler: borrow the idle pp
                            # psum banks to deepen the S pipeline
                            stag = "pp" if (last_unit and idx % 2 == 1) else "s"
                            s_ps = ps.tile([P, NI], f32, tag=stag, bufs=2, name="s_ps")
                            nc.tensor.matmul(
                                s_ps[:, :w],
                                kh[:, P * jt : P * (jt + 1)],
                                qh[:, NI * ic : NI * ic + w],
                                start=True,
                                stop=True,
                            )
                            crossing = (ic == 0 and jt < 4) or (ic == 1 and jt >= 4)
                            if crossing:
                                nc.vector.tensor_tensor(
                                    s_ps[:, :w], s_ps[:, :w], ad[jt][:, :w], ADD
                                )
                            e_sb = sb.tile([P, NI], mdt, tag="e", bufs=6, name="e_sb")
                            nc.scalar.activation(e_sb[:, :w], s_ps[:, :w], Exp)
                            pend.append((idx, jt, w, e_sb))
                            # unit (0,0) fills v_g concurrently (proj_v filler):
                            # one extra pipeline step so v[jts[0]]=v3 is emitted
                            # before the first AV drain reads it.
                            lag = 4 if (g == 0 and ot == 0 and ic == 0) else 3
                            while len(pend) > lag:
                                drain_one()
                            yield
                        while pend:
                            drain_one()
                        nc.tensor.matmul(
                            c_ps[:], ones[:], eacc[0][:], start=True, stop=True
                        )

                        if ic == 1:
                            # fold the j<512 contributions of column i=1023 in
                            for jc in range(4):
                                nc.tensor.matmul(
                                    col_ps[:, 4:5],
                                    smallmm(v_g[jc][:, P * ot : P * (ot + 1)]),
                                    smallmm(colE[:, jc : jc + 1]),
                                    start=(jc == 0),
                                    stop=(jc == 3),
                                )
                            # sum the 4 per-tile exception exps on DVE (free-dim
                            # reduce), then one 1-col ones-matmul broadcasts the
                            # partition sum — replaces 4 tiny matmuls with 1.
                            colR = sb.tile(
                                [P, 1],
                                f32 if use_f32r else mdt,
                                tag="colR",
                                bufs=2,
                                name="colR",
                            )
                            nc.vector.reduce_sum(
                                colR[:],
                                smallmm(colE[:, 0:4]),
                                axis=mybir.AxisListType.X,
                            )
                            nc.tensor.matmul(
                                col_ps[:, 5:6],
                                smallmm(ones[:]),
                                colR[:],
                                start=True,
                                stop=True,
                            )
                            colsb = sb.tile([P, 2], f32, tag="colsb", bufs=2, name="colsb")
                            nc.scalar.activation(colsb[:], col_ps[:, 4:6], Copy)
                            # column 1023 sits at window position 0 (rotated)
                            nc.vector.tensor_tensor(
                                u_ps[:, 0:1], u_ps[:, 0:1], colsb[:, 0:1], ADD
                            )
                            nc.vector.tensor_tensor(
                                c_ps[:, 0:1], c_ps[:, 0:1], colsb[:, 1:2], ADD
                            )

                        recip = sb.tile([P, NI], f32, tag="recip", bufs=2, name="recip")
                        o_sb = sb.tile([P, NI], f32, tag="o", bufs=3, name="o_sb")
                        if last_unit and ic == 0:
                            # final epilogue is fully exposed: halve the DVE
                            # chain so the first out-DMA overlaps the second
                            hn = NI // 2
                            for hf in range(2):
                                sl = slice(hn * hf, hn * (hf + 1))
                                nc.vector.reciprocal(recip[:, sl], c_ps[:, sl])
                                nc.vector.tensor_tensor(
                                    o_sb[:, sl], u_ps[:, sl], recip[:, sl], MUL
                                )
                                nc.sync.dma_start(
                                    out_d.ap()[
                                        P * h : P * (h + 1),
                                        NI * ic + hn * hf : NI * ic + hn * (hf + 1),
                                    ],
                                    o_sb[:, sl],
                                )
                        else:
                            nc.vector.reciprocal(recip[:], c_ps[:])
                            nc.vector.tensor_tensor(o_sb[:], u_ps[:], recip[:], MUL)
                            nc.sync.dma_start(
                                out_d.ap()[P * h : P * (h + 1), NI * ic : NI * (ic + 1)],
                                o_sb[:],
                            )
                        yield

                # ---------------- schedule ----------------
                load_weights(0)
                ones = sb.tile([P, P], mdt, tag="ones", name="ones")
                nc.gpsimd.dma_start(ones[:], on_d.ap()[:])
                for jt in range(4, 8):  # ic=1 crossing blocks, needed later
                    t = sb.tile([P, NI], f32, tag=f"ad{jt}", name=f"ad{jt}")
                    nc.gpsimd.dma_start(t[:], ad_d.ap()[:, NI * jt : NI * (jt + 1)])
                    ad[jt] = t
                for _ in proj_qk_gen(0, 0, copy_eng=nc.vector):
                    pass

                # Filler generators are window-scoped: proj work for (g1, ot)
                # may only be emitted strictly after attn(g0, ot) has finished
                # emitting (WAR hazards on the single-buffered qT/kT/w tiles
                # would otherwise deadlock the in-order engine queues).
                windows = {
                    (0, 0): [proj_v_gen(0), proj_qk_gen(0, 1)],
                    (0, 1): [proj_qk_gen(0, 2)],
                    (0, 2): [proj_qk_gen(0, 3), wload_gen(1), proj_qk_gen(1, 0)],
                    (0, 3): [proj_v_gen(1)],
                    (1, 0): [proj_qk_gen(1, 1)],
                    (1, 1): [proj_qk_gen(1, 2)],
                    (1, 2): [proj_qk_gen(1, 3)],
                }

                for g in range(NG):
                    for ot in range(GH):
                        filler = deque(windows.get((g, ot), []))

                        def pump(n):
                            while n > 0 and filler:
                                try:
                                    next(filler[0])
                                    n -= 1
                                except StopIteration:
                                    filler.popleft()

                        for _ in attn_gen(g, ot):
                            pump(1)
                        pump(10**9)  # drain before the next unit starts

            for _rep in range(reps):
                emit()

    if compile:
        nc.compile()
    return nc


def _get_program():
    global _PROGRAM
    if _PROGRAM is None:
        _PROGRAM = build_program()
    return _PROGRAM


def make_in_maps(x, Wq, Wk, Wv):
    scale = 1.0 / np.sqrt(np.float32(DH))
    wqT = np.ascontiguousarray(np.asarray(Wq, np.float32).T * scale)
    wkT = np.ascontiguousarray(np.asarray(Wk, np.float32).T)
    wvT = np.ascontiguousarray(np.asarray(Wv, np.float32).T)
    adders = _adder_patterns()
    ones = np.ones((P, P), np.float32)
    x = np.asarray(x, np.float32)
    if MM_DT == "bf16":
        import ml_dtypes

        bf16 = ml_dtypes.bfloat16
        wqT, wkT, wvT = wqT.astype(bf16), wkT.astype(bf16), wvT.astype(bf16)
        ones = ones.astype(bf16)
        x = x.astype(bf16)
    in_maps = []
    for b in range(B):
        in_maps.append(
            {
                "xT": np.ascontiguousarray(x[b].T),
                "wqT": wqT,
                "wkT": wkT,
                "wvT": wvT,
                "adders": adders,
                "ones_t": ones,
            }
        )
    return in_maps


def unpermute_out(out_btd: np.ndarray) -> np.ndarray:
    """Undo the device's rotated ic=1 window: token positions [512..1024) are
    stored as [1023, 512, 513, .., 1022]; roll them back to natural order."""
    out_btd[:, NI:, :] = np.roll(out_btd[:, NI:, :], -1, axis=1)
    return out_btd


def kernel(x, mask, Wq, Wk, Wv, _trace=False):
    from concourse.bass_utils import run_bass_kernel_spmd

    nc = _get_program()
    in_maps = make_in_maps(x, Wq, Wk, Wv)
    res = run_bass_kernel_spmd(nc, in_maps, core_ids=list(range(B)), trace=_trace)
    out = np.stack([res.results[b]["out"] for b in range(B)], axis=0)
    out = np.swapaxes(out, 1, 2)  # device stores out.T
    out = unpermute_out(np.ascontiguousarray(out))
    out = out * np.asarray(mask, np.float32)[:, :, None]
    out = np.ascontiguousarray(out, np.float32)
    if _trace:
        kernel.last_results = res
    return out



# revision 25
# speedup vs baseline: 310.6537x; 1.0031x over previous
"""Trainium2 Bass kernel for causal ("FORWARD" direction) multi-head attention.

Reference computation (per batch b, n_heads=8, d=128):
  Q = x @ Wq.T ; K = x @ Wk.T ; V = x @ Wv.T          (nn.Linear, no bias)
  scores[h,i,j] = (Qh[i] . Kh[j]) / sqrt(d)
  scores += -10000 where j <= i   (keeps strict upper triangle j > i)
  attn = softmax(scores, axis=j) ; out = attn @ Vh ; concat heads
  Row i=1023 is fully masked; jax softmax's max-subtraction makes it equal
  softmax of the *raw* scores, so the kernel keeps column i=1023 unmasked.

Sharding: data-parallel over batch B=8 -> 8 cores, no collectives.

Device layout (per core, everything transposed so the softmax reduction is a
matmul-friendly partition-dim reduction):
  xT[k,t]       : x.T                                  [1024,1024]
  qT/kT[o,t]    : per head-group of 4 heads            via Wq.T/Wk.T as lhsT
  v[t,o]        : natural V                            via xT as lhsT
  S_T[j,i]      = kT_tile.T @ qT  (contraction over d=128, single tile)
  expS          = exp(S_T + adder)   (adder patterns precomputed on host)
  U_T[dd,i]     = sum_j V[j,dd] expS[j,i]   (matmul accum over j tiles)
  colsum[*,i]   = ones.T @ expS             (partition-broadcast row of sums)
  out_T         = U_T * reciprocal(colsum) -> DRAM (host transposes back)

Scheduling: projections for the NEXT head-group are emitted interleaved with
attention of the current group, so PE fills the gaps where it would otherwise
wait on Activation (exp) results.

Performance notes (HW-measured via amortized reps-in-NEFF slope):
  - This hardware streams matmul moving operands at 1 column/cycle @
    ~1.28 GHz for f32r AND bf16 alike (no warm 2.4 GHz state, no 2x bf16
    column rate).  Measured: [128,128]@[128,512] MM = 417.6 ns = pure
    512-column streaming; N=256 -> 259 ns, N=128 -> 287 ns (large fixed
    per-MM overhead below N=512).
  - The all-N=512 checkpoint (592 MMs = 303K moving columns) measured
    237.2 us = its column-streaming floor (zero PE bubbles).  SPLITTING
    live blocks finer (N=256/128) loses to the per-MM overhead, but
    NARROWING each crossing tile's moving width to its live prefix cuts
    dead columns at unchanged MM count: ic=0 tile jt is live only for
    i < 128*(jt+1); rotating the ic=1 window (column 1023 stored first,
    un-rotated host-side by unpermute_out) makes ic=1 tiles live-prefix
    contiguous too.  Measured: 237.2 -> ~218 us slope (223-230 us fully
    amortized incl. per-execute overhead), via narrow ic=0 + ic=1 +
    folding the 4 csum-fold matmuls into 1 via a DVE reduce.  fp8
    (DoubleRow) loses to numerics (3.6% per-element noise does not
    average out in random-sign sums -> ~5% output error vs the 2e-2
    gate); bf16 gains nothing (same column rate) and adds error, hence
    f32r.  Offloading colsum to gpsimd partition_all_reduce passes
    CoreSim but WEDGES the device (NRT_EXEC_UNIT_UNRECOVERABLE) — do
    not retry.  No LDWEIGHTS elision exists (identical-stationary
    back-to-back matmuls cost the same as rotating), so stationary-
    sharing restructures gain nothing.
"""

import os
import sys
from collections import deque

import numpy as np

if "/opt/trn_rl_repo" not in sys.path:
    sys.path.insert(0, "/opt/trn_rl_repo")

B, T, D, H, DH = 8, 1024, 1024, 8, 128
P = 128          # partition tile
NI = 512         # i-chunk (moving free size)
NG, GH = 2, 4    # head groups x heads per group
NKT = T // P     # 8 contraction tiles
# f32r (fp32 bits, full-rate matmul streaming) is the default: measured on
# this hardware, bf16 matmuls stream at the same 1 column/cycle rate as
# f32r (no 2x), so bf16 would only add quantization error.
MM_DT = os.environ.get("KERNEL_MM_DT", "f32r")  # f32r | f32 | bf16

_PROGRAM = None  # cached compiled Bass program


def _adder_patterns() -> np.ndarray:
    """[128, 8*512] f32. Blocks 0..3: adders for tiles (jt, ic=0); blocks
    4..7: adders for tiles (jt, ic=1) in the ROTATED ic=1 window order
    (position 0 = column i=1023 kept raw, positions 1..511 = columns
    512..1022).  The rotation puts the exception column inside every
    narrowed tile's live prefix."""
    ad = np.zeros((P, 8, NI), np.float32)
    j = np.arange(P)
    i = np.arange(NI)
    for jt in range(4):
        ad[:, jt, :] = np.where((P * jt + j)[:, None] <= i[None, :], -10000.0, 0.0)
    gi = np.concatenate([[T - 1], NI + np.arange(NI - 1)])  # window pos -> i
    for jt in range(4, 8):
        blk = np.where((P * jt + j)[:, None] <= gi[None, :], -10000.0, 0.0)
        blk[:, 0] = 0.0  # column i=1023 stays raw
        ad[:, jt, :] = blk
    return np.ascontiguousarray(ad.reshape(P, 8 * NI))


def build_program(mm_dt: str = MM_DT, compile: bool = True, reps: int = 1):
    import concourse.bass as bass  # noqa: F401
    import concourse.tile as tile
    from concourse import bacc, mybir

    f32 = mybir.dt.float32
    use_f32r = mm_dt == "f32r"
    mdt = {
        "bf16": mybir.dt.bfloat16,
        "f32r": mybir.dt.float32r,
        "f32": mybir.dt.float32,
    }[mm_dt]
    Exp = mybir.ActivationFunctionType.Exp
    Copy = mybir.ActivationFunctionType.Copy
    ADD = mybir.AluOpType.add
    MUL = mybir.AluOpType.mult

    nc = bacc.Bacc(
        "TRN2",
        target_bir_lowering=False,
        debug=False,
        enable_asserts=False,
        num_devices=B,
    )

    xT_d = nc.dram_tensor("xT", [D, T], mdt, kind="ExternalInput")
    wq_d = nc.dram_tensor("wqT", [D, D], mdt, kind="ExternalInput")
    wk_d = nc.dram_tensor("wkT", [D, D], mdt, kind="ExternalInput")
    wv_d = nc.dram_tensor("wvT", [D, D], mdt, kind="ExternalInput")
    ad_d = nc.dram_tensor("adders", [P, 8 * NI], f32, kind="ExternalInput")
    on_d = nc.dram_tensor("ones_t", [P, P], mdt, kind="ExternalInput")
    # out is stored TRANSPOSED ([D, T]); the host wrapper transposes back.
    out_d = nc.dram_tensor("out", [D, T], f32, kind="ExternalOutput")

    with tile.TileContext(nc) as tc:
        with (
            tc.tile_pool(name="sb", bufs=1) as sb,
            tc.tile_pool(name="ps", bufs=1, space="PSUM") as ps,
        ):
            KT_ORDER = (4, 0, 5, 1, 6, 2, 7, 3)

            def emit():
                # ---------------- resident loads ----------------
                # xT split across the SP and ACT DMA queues; weights on
                # Pool/SP; adder blocks trickle in on Pool in first-use
                # order.  kt loops consume in KT_ORDER = arrival order.
                xT = [None] * NKT

                def load_xT(k, eng):
                    t = sb.tile([P, T], mdt, tag=f"xT{k}", name=f"xT{k}")
                    eng.dma_start(t[:], xT_d.ap()[P * k : P * (k + 1), :])
                    xT[k] = t

                for k in (4, 0, 5, 1):
                    load_xT(k, nc.sync)
                for k in (6, 2, 7, 3):
                    load_xT(k, nc.scalar)
                ad = [None] * 8
                for jt in range(4):  # ic=0 crossing blocks, needed first
                    t = sb.tile([P, NI], f32, tag=f"ad{jt}", name=f"ad{jt}")
                    nc.scalar.dma_start(t[:], ad_d.ap()[:, NI * jt : NI * (jt + 1)])
                    ad[jt] = t

                copy_flip = [0]

                def psum_to_sbuf(dst_ap, src_ap, eng=None):
                    if eng is None:
                        eng = nc.scalar if copy_flip[0] % 2 == 0 else nc.vector
                        copy_flip[0] += 1
                    if eng is nc.scalar:
                        eng.activation(dst_ap, src_ap, Copy)
                    else:
                        eng.tensor_copy(dst_ap, src_ap)


                W, QKV = {}, {}

                def load_weights(g):
                    dram = {"wq": wq_d, "wk": wk_d, "wv": wv_d}
                    if g == 0:
                        placement = {
                            "wq": [(kt, nc.gpsimd) for kt in KT_ORDER],
                            "wk": [(4, nc.sync), (0, nc.sync), (5, nc.sync),
                                   (1, nc.sync), (6, nc.gpsimd), (2, nc.gpsimd),
                                   (7, nc.gpsimd), (3, nc.gpsimd)],
                            "wv": [(4, nc.sync), (0, nc.sync), (5, nc.sync),
                                   (1, nc.sync), (6, nc.gpsimd), (2, nc.gpsimd),
                                   (7, nc.gpsimd), (3, nc.gpsimd)],
                        }
                        order = ("wq", "wk", "wv")
                    else:
                        placement = {
                            nm: [(kt, nc.sync) for kt in KT_ORDER]
                            for nm in ("wv", "wq", "wk")
                        }
                        order = ("wv", "wq", "wk")
                    lists = {}
                    for nm in order:
                        lst = [None] * NKT
                        for kt, eng in placement[nm]:
                            w = sb.tile(
                                [P, NI], mdt, tag=f"{nm}{kt}", name=f"{nm}{kt}g{g}"
                            )
                            eng.dma_start(
                                w[:],
                                dram[nm].ap()[P * kt : P * (kt + 1), NI * g : NI * (g + 1)],
                            )
                            lst[kt] = w
                        lists[nm] = lst
                    W[g] = (lists["wq"], lists["wk"], lists["wv"])
                    QKV[g] = (
                        [
                            sb.tile([P, T], mdt, tag=f"qT{ot}", name=f"qT{ot}g{g}")
                            for ot in range(GH)
                        ],
                        [
                            sb.tile([P, T], mdt, tag=f"kT{ot}", name=f"kT{ot}g{g}")
                            for ot in range(GH)
                        ],
                        [
                            sb.tile([P, NI], mdt, tag=f"v{tt}", bufs=2, name=f"v{tt}g{g}")
                            for tt in range(NKT)
                        ],
                    )

                # ---------------- projection generators ----------------
                def proj_qk_gen(g, ot, copy_eng=None):
                    wq_g, wk_g, _ = W[g]
                    qT_g, kT_g, _ = QKV[g]
                    for wlist, dst in ((wq_g, qT_g[ot]), (wk_g, kT_g[ot])):
                        is_q = wlist is wq_g
                        for tci in range(2):
                            pp = ps.tile([P, NI], f32, tag="pp", bufs=2, name="pp")
                            for ki, kt in enumerate(KT_ORDER):
                                nc.tensor.matmul(
                                    pp[:],
                                    wlist[kt][:, P * ot : P * (ot + 1)],
                                    xT[kt][:, NI * tci : NI * (tci + 1)],
                                    start=(ki == 0),
                                    stop=(ki == NKT - 1),
                                )
                            if is_q and tci == 1:
                                # store qT's second half ROTATED by one: column
                                # 1023 lands at position 512 so the ic=1 window
                                # reads [1023, 512, 513, ...] contiguously.
                                psum_to_sbuf(
                                    dst[:, NI + 1 : T], pp[:, 0 : NI - 1], copy_eng
                                )
                                psum_to_sbuf(
                                    dst[:, NI : NI + 1], pp[:, NI - 1 : NI], copy_eng
                                )
                            else:
                                psum_to_sbuf(
                                    dst[:, NI * tci : NI * (tci + 1)], pp[:], copy_eng
                                )
                            yield

                def proj_v_gen(g):
                    _, _, wv_g = W[g]
                    _, _, v_g = QKV[g]
                    for tt in range(NKT):
                        pp = ps.tile([P, NI], f32, tag="pp", bufs=2, name="pp")
                        for ki, kt in enumerate(KT_ORDER):
                            nc.tensor.matmul(
                                pp[:],
                                xT[kt][:, P * tt : P * (tt + 1)],
                                wv_g[kt][:],
                                start=(ki == 0),
                                stop=(ki == NKT - 1),
                            )
                        psum_to_sbuf(v_g[tt][:], pp[:])
                        yield

                def wload_gen(g):
                    load_weights(g)
                    return
                    yield  # noqa: unreachable - makes this a generator

                # ---------------- attention generator ----------------
                def attn_gen(g, ot):
                    h = GH * g + ot
                    last_unit = g == NG - 1 and ot == GH - 1
                    qT_g, kT_g, v_g = QKV[g]
                    qh, kh = qT_g[ot], kT_g[ot]

                    # HW rejects fp32r matmuls with tiny output free size
                    # (s3d3_mm_fp32r_restrictions); run those as plain fp32.
                    def smallmm(ap):
                        return ap.bitcast(f32) if use_f32r else ap

                    # last unit runs ic=1 first: ic=0 has no exception path,
                    # so the end-of-program dependency tail is shorter
                    ic_order = (1, 0) if last_unit else (0, 1)
                    for ic in ic_order:
                        # Full-width tiles drain first so the start AV matmul
                        # covers the whole PSUM bank (narrowed tiles then
                        # pure-accumulate into already-written regions); for
                        # unit (0,0) the ic=0 order also keeps jts[k] <= k+3+1
                        # = the proj_v filler's progress at that drain.
                        jts = [3, 4, 5, 6, 0, 1, 2, 7] if ic == 0 else [7, 4, 5, 6]
                        nj = len(jts)

                        u_ps = ps.tile([P, NI], f32, tag="u", bufs=2, name="u_ps")
                        c_ps = ps.tile([P, NI], f32, tag="c", bufs=1, name="c_ps")

                        col_ps = colE = None
                        if ic == 1:
                            # raw scores for column i=1023, rows j in [0,512);
                            # that column is stored at position NI (rotated)
                            col_ps = ps.tile([P, 8], f32, tag="col", bufs=1, name="col_ps")
                            for jc in range(4):
                                nc.tensor.matmul(
                                    col_ps[:, jc : jc + 1],
                                    smallmm(kh[:, P * jc : P * (jc + 1)]),
                                    smallmm(qh[:, NI : NI + 1]),
                                    start=True,
                                    stop=True,
                                )
                            colE = sb.tile([P, 8], mdt, tag="colE", bufs=2, name="colE")
                            nc.scalar.activation(colE[:, 0:4], col_ps[:, 0:4], Exp)

                        pend = []
                        eacc = [None]

                        def drain_one():
                            idx, jt, w, e_sb = pend.pop(0)
                            first, last = idx == 0, idx == nj - 1
                            nc.tensor.matmul(
                                u_ps[:, :w],
                                v_g[jt][:, P * ot : P * (ot + 1)],
                                e_sb[:, :w],
                                start=first,
                                stop=last,
                            )
                            # colsum via elementwise tile accumulation (Pool);
                            # one ones-matmul at the end reduces partitions.
                            # SBUF-only chain -> Pool (gpsimd can't touch PSUM).
                            # jts ordering guarantees idx 0 and 1 are full-width
                            # tiles, so the alias/alloc chain stays exact; the
                            # narrowed tiles accumulate in place on their slice.
                            if idx == 0:
                                eacc[0] = e_sb
                            elif idx == 1:
                                acc = sb.tile(
                                    [P, NI], mdt, tag="eacc", bufs=2, name="eacc"
                                )
                                if w == NI:
                                    nc.gpsimd.tensor_tensor(
                                        acc[:], eacc[0][:], e_sb[:], ADD
                                    )
                                else:
                                    nc.gpsimd.tensor_tensor(
                                        acc[:, :w], eacc[0][:, :w], e_sb[:, :w], ADD
                                    )
                                    nc.gpsimd.tensor_copy(
                                        acc[:, w:], eacc[0][:, w:]
                                    )
                                eacc[0] = acc
                            else:
                                nc.gpsimd.tensor_tensor(
                                    eacc[0][:, :w], eacc[0][:, :w], e_sb[:, :w], ADD
                                )

                        for idx, jt in enumerate(jts):
                            # Crossing tiles are dead beyond their diagonal:
                            # ic=0 tile jt is live only for i < 128*(jt+1);
                            # ic=1 tile jt (rotated window) only for positions
                            # < 128*(jt-3) (incl. pos 0 = raw column 1023).
                            # Narrow the moving width so S/exp/AV skip the
                            # dead columns.
                            if ic == 0:
                                w = min(P * (jt + 1), NI)
                            else:
                                w = NI if jt == 7 else P * (jt - 3)
                            # last unit has no proj filler: borrow the idle pp
                            # psum banks to deepen the S pipeline
                            stag = "pp" if (last_unit and idx % 2 == 1) else "s"
                            s_ps = ps.tile([P, NI], f32, tag=stag, bufs=2, name="s_ps")
                            nc.tensor.matmul(
                                s_ps[:, :w],
                                kh[:, P * jt : P * (jt + 1)],
                                qh[:, NI * ic : NI * ic + w],
                                start=True,
                                stop=True,
                            )
                            crossing = (ic == 0 and jt < 4) or (ic == 1 and jt >= 4)
                            if crossing:
                                nc.vector.tensor_tensor(
                                    s_ps[:, :w], s_ps[:, :w], ad[jt][:, :w], ADD
                                )
                            e_sb = sb.tile([P, NI], mdt, tag="e", bufs=6, name="e_sb")
                            nc.scalar.activation(e_sb[:, :w], s_ps[:, :w], Exp)
                            pend.append((idx, jt, w, e_sb))
                            # unit (0,0) fills v_g concurrently (proj_v filler):
                            # one extra pipeline step so v[jts[0]]=v3 is emitted
                            # before the first AV drain reads it.
                            lag = 4 if (g == 0 and ot == 0 and ic == 0) else 3
                            while len(pend) > lag:
                                drain_one()
                            yield
                        while pend:
                            drain_one()
                        nc.tensor.matmul(
                            c_ps[:], ones[:], eacc[0][:], start=True, stop=True
                        )

                        if ic == 1:
                            # fold the j<512 contributions of column i=1023 in
                            for jc in range(4):
                                nc.tensor.matmul(
                                    col_ps[:, 4:5],
                                    smallmm(v_g[jc][:, P * ot : P * (ot + 1)]),
                                    smallmm(colE[:, jc : jc + 1]),
                                    start=(jc == 0),
                                    stop=(jc == 3),
                                )
                            # sum the 4 per-tile exception exps on DVE (free-dim
                            # reduce), then one 1-col ones-matmul broadcasts the
                            # partition sum — replaces 4 tiny matmuls with 1.
                            colR = sb.tile(
                                [P, 1],
                                f32 if use_f32r else mdt,
                                tag="colR",
                                bufs=2,
                                name="colR",
                            )
                            nc.vector.reduce_sum(
                                colR[:],
                                smallmm(colE[:, 0:4]),
                                axis=mybir.AxisListType.X,
                            )
                            nc.tensor.matmul(
                                col_ps[:, 5:6],
                                smallmm(ones[:]),
                                colR[:],
                                start=True,
                                stop=True,
                            )
                            colsb = sb.tile([P, 2], f32, tag="colsb", bufs=2, name="colsb")
                            nc.scalar.activation(colsb[:], col_ps[:, 4:6], Copy)
                            # column 1023 sits at window position 0 (rotated)
                            nc.vector.tensor_tensor(
                                u_ps[:, 0:1], u_ps[:, 0:1], colsb[:, 0:1], ADD
                            )
                            nc.vector.tensor_tensor(
                                c_ps[:, 0:1], c_ps[:, 0:1], colsb[:, 1:2], ADD
                            )

                        recip = sb.tile([P, NI], f32, tag="recip", bufs=2, name="recip")
                        o_sb = sb.tile([P, NI], f32, tag="o", bufs=3, name="o_sb")
                        if last_unit and ic == 0:
                            # final epilogue is fully exposed: halve the DVE
                            # chain so the first out-DMA overlaps the second
                            hn = NI // 2
                            for hf in range(2):
                                sl = slice(hn * hf, hn * (hf + 1))
                                nc.vector.reciprocal(recip[:, sl], c_ps[:, sl])
                                nc.vector.tensor_tensor(
                                    o_sb[:, sl], u_ps[:, sl], recip[:, sl], MUL
                                )
                                nc.sync.dma_start(
                                    out_d.ap()[
                                        P * h : P * (h + 1),
                                        NI * ic + hn * hf : NI * ic + hn * (hf + 1),
                                    ],
                                    o_sb[:, sl],
                                )
                        else:
                            nc.vector.reciprocal(recip[:], c_ps[:])
                            nc.vector.tensor_tensor(o_sb[:], u_ps[:], recip[:], MUL)
                            nc.sync.dma_start(
                                out_d.ap()[P * h : P * (h + 1), NI * ic : NI * (ic + 1)],
                                o_sb[:],
                            )
                        yield

                # ---------------- schedule ----------------
                load_weights(0)
                ones = sb.tile([P, P], mdt, tag="ones", name="ones")
                nc.gpsimd.dma_start(ones[:], on_d.ap()[:])
                for jt in range(4, 8):  # ic=1 crossing blocks, needed later
                    t = sb.tile([P, NI], f32, tag=f"ad{jt}", name=f"ad{jt}")
                    nc.gpsimd.dma_start(t[:], ad_d.ap()[:, NI * jt : NI * (jt + 1)])
                    ad[jt] = t
                for _ in proj_qk_gen(0, 0, copy_eng=nc.vector):
                    pass

                # Filler generators are window-scoped: proj work for (g1, ot)
                # may only be emitted strictly after attn(g0, ot) has finished
                # emitting (WAR hazards on the single-buffered qT/kT/w tiles
                # would otherwise deadlock the in-order engine queues).
                windows = {
                    (0, 0): [proj_v_gen(0), proj_qk_gen(0, 1)],
                    (0, 1): [proj_qk_gen(0, 2)],
                    (0, 2): [proj_qk_gen(0, 3), wload_gen(1), proj_qk_gen(1, 0)],
                    (0, 3): [proj_v_gen(1)],
                    (1, 0): [proj_qk_gen(1, 1)],
                    (1, 1): [proj_qk_gen(1, 2)],
                    (1, 2): [proj_qk_gen(1, 3)],
                }

                for g in range(NG):
                    for ot in range(GH):
                        filler = deque(windows.get((g, ot), []))

                        def pump(n):
                            while n > 0 and filler:
                                try:
                                    next(filler[0])
                                    n -= 1
                                except StopIteration:
                                    filler.popleft()

                        for _ in attn_gen(g, ot):
                            pump(1)
                        pump(10**9)  # drain before the next unit starts

            for _rep in range(reps):
                emit()

    if compile:
        nc.compile()
    return nc


def _get_program():
    global _PROGRAM
    if _PROGRAM is None:
        _PROGRAM = build_program()
    return _PROGRAM


def make_in_maps(x, Wq, Wk, Wv):
    scale = 1.0 / np.sqrt(np.float32(DH))
    wqT = np.ascontiguousarray(np.asarray(Wq, np.float32).T * scale)
    wkT = np.ascontiguousarray(np.asarray(Wk, np.float32).T)
    wvT = np.ascontiguousarray(np.asarray(Wv, np.float32).T)
    adders = _adder_patterns()
    ones = np.ones((P, P), np.float32)
    x = np.asarray(x, np.float32)
    if MM_DT == "bf16":
        import ml_dtypes

        bf16 = ml_dtypes.bfloat16
        wqT, wkT, wvT = wqT.astype(bf16), wkT.astype(bf16), wvT.astype(bf16)
        ones = ones.astype(bf16)
        x = x.astype(bf16)
    in_maps = []
    for b in range(B):
        in_maps.append(
            {
                "xT": np.ascontiguousarray(x[b].T),
                "wqT": wqT,
                "wkT": wkT,
                "wvT": wvT,
                "adders": adders,
                "ones_t": ones,
            }
        )
    return in_maps


def unpermute_out(out_btd: np.ndarray) -> np.ndarray:
    """Undo the device's rotated ic=1 window: token positions [512..1024) are
    stored as [1023, 512, 513, .., 1022]; roll them back to natural order."""
    out_btd[:, NI:, :] = np.roll(out_btd[:, NI:, :], -1, axis=1)
    return out_btd


def kernel(x, mask, Wq, Wk, Wv, _trace=False):
    from concourse.bass_utils import run_bass_kernel_spmd

    nc = _get_program()
    in_maps = make_in_maps(x, Wq, Wk, Wv)
    res = run_bass_kernel_spmd(nc, in_maps, core_ids=list(range(B)), trace=_trace)
    out = np.stack([res.results[b]["out"] for b in range(B)], axis=0)
    out = np.swapaxes(out, 1, 2)  # device stores out.T
    out = unpermute_out(np.ascontiguousarray(out))
    out = out * np.asarray(mask, np.float32)[:, :, None]
    out = np.ascontiguousarray(out, np.float32)
    if _trace:
        kernel.last_results = res
    return out

